# revision 1
# baseline (speedup 1.0000x reference)
import sys
import os

sys.path.insert(0, "/opt/trn_rl_repo")

import numpy as np
import ml_dtypes

import concourse.bass as bass
import concourse.mybir as mybir
import concourse.tile as tile
from concourse import bacc
from concourse.bass import IndirectOffsetOnAxis
from concourse.bass_utils import run_bass_kernel_spmd

BF16 = ml_dtypes.bfloat16

# model dims (fixed by the problem)
SITE_PROPS = 92
SITE_EMB = 64
BOND_EMB = 64
BOND_EXP = 64
MAX_DIST = 8.0
H1, H2, OUT = 128, 64, 1
N_GRAPHS = 512
GCHUNKS = N_GRAPHS // 128

F32 = mybir.dt.float32
BF = mybir.dt.bfloat16
I32 = mybir.dt.int32

STEP = MAX_DIST / BOND_EXP
EXP_SCALE = -1.0 / (STEP * STEP)


class Cfg:
    def __init__(self, n_cores, nblk, T, n_sites, n_graphs=N_GRAPHS,
                 gate_bias=False):
        self.n_cores = n_cores
        self.nblk = nblk          # 128-site blocks per core
        self.T = T                # edge tiles (128 edges) per block, even
        self.R = nblk * 128       # sites per core
        self.site_pad = n_cores * self.R
        self.n_sites = n_sites
        self.n_graphs = n_graphs
        self.gate_bias = gate_bias
        self.phases = "EBC12PH"  # for bisection


def build_graph_kernel(nc, tc, ins, outs, cfg):
    """Build the full CGCNN forward pass. ins/outs: dicts of DRAM APs."""
    NBLK, T, R = cfg.nblk, cfg.T, cfg.R
    NC = cfg.n_cores
    PAIRS = T // 2
    # activation-group sizes for batching sigmoid/relu over psum bank (<=8 tiles)
    groups = []
    t0 = 0
    while t0 < T:
        g = min(8, T - t0)
        groups.append((t0, g))
        t0 += g

    from contextlib import ExitStack
    stack = ExitStack()
    dram = stack.enter_context(tc.tile_pool(name="dram", bufs=1, space="DRAM"))
    tabA = dram.tile([cfg.site_pad, SITE_EMB], BF, addr_space="Shared")
    tabB = dram.tile([cfg.site_pad, SITE_EMB], BF, addr_space="Shared")
    emb_slice = dram.tile([R, SITE_EMB], BF)
    l1_slice = dram.tile([R, SITE_EMB], BF)
    l2_slice = dram.tile([R, SITE_EMB], BF)
    beTab = dram.tile([NBLK, 128, T * BOND_EMB], BF)     # transposed bond embeds
    pool_part = dram.tile([cfg.n_graphs, SITE_EMB + 1], F32)
    pool_full = dram.tile([cfg.n_graphs, SITE_EMB + 1], F32,
                          addr_space="Shared")

    cp = stack.enter_context(tc.tile_pool(name="consts", bufs=1))

    def load_const(name, shape, dtype):
        t = cp.tile(shape, dtype, name=f"c_{name}", tag=f"c_{name}")
        nc.sync.dma_start(t[:], ins[name][:])
        return t

    identity = load_const("identity128", [128, 128], F32)
    identity_bf = load_const("identity128_bf", [128, 128], BF)
    iota128 = load_const("iota_row128", [128, 128], F32)
    iota512 = load_const("iota512", [128, cfg.n_graphs], F32)
    centers = load_const("centers_row", [128, BOND_EXP], F32)
    ones_bf = load_const("ones_col", [128, 1], BF)
    Wse = load_const("Wse", [SITE_PROPS, SITE_EMB], F32)
    bse = load_const("bse", [SITE_EMB, 1], F32)
    Wbe_diag = load_const("Wbe_diag", [128, 128], BF)
    bbe2 = load_const("bbe2", [128, 1], F32)
    Wtop = {}
    Wbot2 = {}
    for L in (1, 2):
        for gate in ("sig", "sof"):
            Wtop[(L, gate)] = load_const(f"W{gate}{L}_top", [128, SITE_EMB], BF)
            Wbot2[(L, gate)] = load_const(f"W{gate}{L}_bot2", [128, SITE_EMB], BF)
    bgate = {}
    if cfg.gate_bias:
        for L in (1, 2):
            for gate in ("sig", "sof"):
                # bias replicated along free dim for all 8 tiles of a group
                bgate[(L, gate)] = load_const(f"b{gate}{L}_row", [128, 8 * 64], F32)
    W1 = load_const("W1", [SITE_EMB, H1], F32)
    b1 = load_const("b1", [H1, 1], F32)
    W2 = load_const("W2", [H1, H2], F32)
    b2 = load_const("b2", [H2, 1], F32)
    W3 = load_const("W3", [H2, OUT], F32)
    b3 = load_const("b3", [1, 1], F32)

    # ---------------- Phase E: site embedding (own slice) ----------------
    if "E" in cfg.phases:
      with (
        tc.tile_pool(name="emb_sb", bufs=3) as esb,
        tc.tile_pool(name="emb_ps", bufs=2, space="PSUM") as eps,
    ):
        for b in range(NBLK):
            srow = esb.tile([128, SITE_PROPS], F32, tag="srow")
            nc.sync.dma_start(srow[:], ins["sites_slice"][b * 128:(b + 1) * 128, :])
            stp = eps.tile([SITE_PROPS, 128], F32, tag="stp", space="PSUM")
            nc.tensor.transpose(stp[:], srow[:], identity[:])
            sts = esb.tile([SITE_PROPS, 128], F32, tag="sts")
            nc.vector.tensor_copy(sts[:], stp[:])
            s0T = eps.tile([SITE_EMB, 128], F32, tag="s0T", space="PSUM")
            nc.tensor.matmul(s0T[:], lhsT=Wse[:], rhs=sts[:], start=True, stop=True)
            s0Tb = esb.tile([SITE_EMB, 128], F32, tag="s0Tb")
            nc.vector.tensor_scalar_add(s0Tb[:], s0T[:], bse[:, 0:1])
            s0p = eps.tile([128, SITE_EMB], F32, tag="s0p", space="PSUM")
            nc.tensor.transpose(s0p[:], s0Tb[:], identity[:SITE_EMB, :SITE_EMB])
            s0row = esb.tile([128, SITE_EMB], BF, tag="s0row")
            nc.vector.tensor_copy(s0row[:], s0p[:])
            nc.sync.dma_start(emb_slice[b * 128:(b + 1) * 128, :], s0row[:])

    if "E" in cfg.phases:
      nc.gpsimd.collective_compute(
        "AllGather", mybir.AluOpType.bypass,
        replica_groups=[list(range(NC))],
        ins=[emb_slice.opt()], outs=[tabA.opt()],
      )

    # ---------------- Phase B: bond embedding (transposed, paired) -------
    if "B" in cfg.phases:
      with (
        tc.tile_pool(name="be_sb", bufs=4) as bsb,
        tc.tile_pool(name="be_ps", bufs=2, space="PSUM") as bps,
    ):
        for b in range(NBLK):
            bonds = bsb.tile([128, T], F32, tag="bonds")
            nc.sync.dma_start(bonds[:], ins["bonds_blk"][b, :, :])
            stage = bsb.tile([128, T * BOND_EMB], BF, tag="beT_stage")
            for q in range(PAIRS):
                bx = bsb.tile([128, 128], BF, tag="bexp_pair")
                for r in range(2):
                    t = 2 * q + r
                    d = bsb.tile([128, BOND_EXP], F32, tag="dmc")
                    nc.vector.tensor_tensor(
                        out=d[:], in0=bonds[:, t:t + 1].to_broadcast([128, BOND_EXP]),
                        in1=centers[:], op=mybir.AluOpType.subtract)
                    nc.vector.tensor_tensor(
                        out=d[:], in0=d[:], in1=d[:], op=mybir.AluOpType.mult)
                    nc.scalar.activation(
                        bx[:, r * 64:(r + 1) * 64], d[:],
                        mybir.ActivationFunctionType.Exp, scale=EXP_SCALE)
                bxTp = bps.tile([128, 128], BF, tag="bexpTp", space="PSUM")
                nc.tensor.transpose(bxTp[:], bx[:], identity_bf[:])
                bxT = bsb.tile([128, 128], BF, tag="bexpT")
                nc.vector.tensor_copy(bxT[:], bxTp[:])
                bep = bps.tile([128, 128], F32, tag="beT_ps", space="PSUM")
                nc.tensor.matmul(bep[:], lhsT=Wbe_diag[:], rhs=bxT[:],
                                 start=True, stop=True)
                nc.vector.tensor_scalar_add(
                    stage[:, q * 128:(q + 1) * 128], bep[:], bbe2[:, 0:1])
            nc.sync.dma_start(beTab[b, :, :], stage[:])

    # ---------------- Phase C: conv layers ----------------
    def conv_layer(L, tab_in, res_in, slice_out):
        with (
            tc.tile_pool(name=f"c{L}_io", bufs=3) as iop,
            tc.tile_pool(name=f"c{L}_g12", bufs=3) as gtp,
            tc.tile_pool(name=f"c{L}_msg", bufs=6) as msb,
            tc.tile_pool(name=f"c{L}_ps", bufs=2, space="PSUM") as cps,
            tc.tile_pool(name=f"c{L}_aps", bufs=1, space="PSUM") as aps,
            tc.tile_pool(name=f"c{L}_tps", bufs=2, space="PSUM") as tps,
        ):
            for b in range(NBLK):
                i1q = iop.tile([128, T * 8], mybir.dt.int16, tag="i1q")
                nc.sync.dma_start(i1q[:], ins["i1q_blk"][b, :, :])
                i2q = iop.tile([128, T * 8], mybir.dt.int16, tag="i2q")
                nc.sync.dma_start(i2q[:], ins["i2q_blk"][b, :, :])
                i1m = iop.tile([128, 3 * T], mybir.dt.uint8, tag="i1m")
                nc.sync.dma_start(i1m[:], ins["i1m_blk"][b, :, :])
                i2m = iop.tile([128, 3 * T], mybir.dt.uint8, tag="i2m")
                nc.sync.dma_start(i2m[:], ins["i2m_blk"][b, :, :])
                rel = iop.tile([128, T], F32, tag="rel")
                nc.sync.dma_start(rel[:], ins["rel_blk"][b, :, :])
                beT = iop.tile([128, T * BOND_EMB], BF, tag="beT")
                nc.sync.dma_start(beT[:], beTab[b, :, :])
                sblk = iop.tile([128, SITE_EMB], BF, tag="sblk")
                nc.sync.dma_start(sblk[:], res_in[b * 128:(b + 1) * 128, :])

                tabq = tab_in[:].rearrange("(a b) c -> a (b c)", b=4)
                q1 = gtp.tile([128, T, 256], BF, tag="q1")
                nc.gpsimd.dma_gather(q1[:], tabq, i1q[:], T * 128, T * 128,
                                     256, single_packet=False)
                q2 = gtp.tile([128, T, 256], BF, tag="q2")
                nc.gpsimd.dma_gather(q2[:], tabq, i2q[:], T * 128, T * 128,
                                     256, single_packet=False)
                s12 = gtp.tile([128, T, 128], BF, tag="s12")
                for (gs, gl) in groups:
                    sl = slice(gs, gs + gl)
                    for (qq, mm, c0) in ((q1, i1m, 0), (q2, i2m, 64)):
                        dst = s12[:, sl, c0:c0 + 64]
                        nc.vector.tensor_copy(dst, qq[:, sl, 0:64])
                        for k in (1, 2, 3):
                            msk = mm[:, (k - 1) * T + gs:(k - 1) * T + gs + gl]
                            nc.vector.copy_predicated(
                                dst, msk.to_broadcast([128, gl, 64]),
                                qq[:, sl, k * 64:(k + 1) * 64])
                if getattr(cfg, "debug", False) and L == 1 and b == 0:
                    nc.sync.dma_start(outs["dbg_s12"][:],
                                      s12[:].rearrange("p t f -> p (t f)"))

                aggT = aps.tile([SITE_EMB, 128], F32, tag="aggT", space="PSUM")
                for (tstart, glen) in groups:
                    gsig = cps.tile([128, 8 * 64], F32, tag="gsig", space="PSUM")
                    gsof = cps.tile([128, 8 * 64], F32, tag="gsof", space="PSUM")
                    for ti in range(glen):
                        t = tstart + ti
                        q, r = divmod(t, 2)
                        s12Tp = tps.tile([128, 128], BF, tag="s12Tp",
                                         space="PSUM")
                        nc.tensor.transpose(s12Tp[:], s12[:, t, :],
                                            identity_bf[:])
                        s12T = msb.tile([128, 128], BF, tag="s12T")
                        nc.vector.tensor_copy(s12T[:], s12Tp[:])
                        if getattr(cfg, "debug", False) and L == 1 and b == 0 and t == 0:
                            nc.sync.dma_start(outs["dbg_s12T"][:], s12T[:])
                        osl = slice(ti * 64, (ti + 1) * 64)
                        for gate, gps in (("sig", gsig), ("sof", gsof)):
                            nc.tensor.matmul(
                                gps[:, osl], lhsT=s12T[:], rhs=Wtop[(L, gate)][:],
                                start=True, stop=False)
                            nc.tensor.matmul(
                                gps[:, osl],
                                lhsT=beT[64 * r:64 * (r + 1),
                                         q * 128:(q + 1) * 128],
                                rhs=Wbot2[(L, gate)][64 * r:64 * (r + 1), :],
                                start=False, stop=True)
                    if cfg.gate_bias:
                        nc.vector.tensor_tensor(
                            out=gsig[:, 0:glen * 64], in0=gsig[:, 0:glen * 64],
                            in1=bgate[(L, "sig")][:, 0:glen * 64],
                            op=mybir.AluOpType.add)
                        nc.vector.tensor_tensor(
                            out=gsof[:, 0:glen * 64], in0=gsof[:, 0:glen * 64],
                            in1=bgate[(L, "sof")][:, 0:glen * 64],
                            op=mybir.AluOpType.add)
                    asig = msb.tile([128, 8 * 64], BF, tag="asig")
                    nc.scalar.activation(
                        asig[:, 0:glen * 64], gsig[:, 0:glen * 64],
                        mybir.ActivationFunctionType.Sigmoid)
                    asof = msb.tile([128, 8 * 64], BF, tag="asof")
                    nc.scalar.activation(
                        asof[:, 0:glen * 64], gsof[:, 0:glen * 64],
                        mybir.ActivationFunctionType.Relu)
                    gmsg = msb.tile([128, 8 * 64], BF, tag="gmsg")
                    nc.vector.tensor_tensor(
                        out=gmsg[:, 0:glen * 64], in0=asig[:, 0:glen * 64],
                        in1=asof[:, 0:glen * 64], op=mybir.AluOpType.mult)
                    if (getattr(cfg, "debug", False) and L == 1 and b == 0
                            and tstart == 0):
                        nc.sync.dma_start(outs["dbg_gmsg"][:], gmsg[:])
                    S8 = msb.tile([128, 8, 128], BF, tag="S8")
                    nc.vector.tensor_tensor(
                        out=S8[:, 0:glen, :],
                        in0=rel[:, tstart:tstart + glen].to_broadcast(
                            [128, glen, 128]),
                        in1=iota128[:].to_broadcast([128, 128, glen]).rearrange(
                            "p d g -> p g d"),
                        op=mybir.AluOpType.is_equal)
                    for ti in range(glen):
                        t = tstart + ti
                        nc.tensor.matmul(
                            aggT[:], lhsT=gmsg[:, ti * 64:(ti + 1) * 64],
                            rhs=S8[:, ti, :], start=(t == 0), stop=(t == T - 1),
                            skip_group_check=True)
                aggTs = msb.tile([SITE_EMB, 128], F32, tag="aggTs")
                nc.vector.tensor_copy(aggTs[:], aggT[:])
                aggp = aps.tile([128, SITE_EMB], F32, tag="aggp", space="PSUM")
                nc.tensor.transpose(aggp[:], aggTs[:],
                                    identity[:SITE_EMB, :SITE_EMB])
                snew = msb.tile([128, SITE_EMB], BF, tag="snew")
                nc.vector.tensor_tensor(
                    out=snew[:], in0=aggp[:], in1=sblk[:],
                    op=mybir.AluOpType.add)
                nc.sync.dma_start(slice_out[b * 128:(b + 1) * 128, :], snew[:])

    if "1" in cfg.phases:
        conv_layer(1, tabA, emb_slice, l1_slice)
        nc.gpsimd.collective_compute(
            "AllGather", mybir.AluOpType.bypass,
            replica_groups=[list(range(NC))],
            ins=[l1_slice.opt()], outs=[tabB.opt()],
        )
    if "2" in cfg.phases:
        conv_layer(2, tabB, l1_slice, l2_slice)

    # ---------------- Phase P: pooling over own sites ----------------
    if "P" in cfg.phases:
      with (
        tc.tile_pool(name="pool_sb", bufs=3) as psb,
        tc.tile_pool(name="pool_ps", bufs=1, space="PSUM") as pps,
    ):
        pool_ps = [
            pps.tile([128, SITE_EMB + 1], F32, tag=f"pool{c}", space="PSUM",
                     name=f"pool_ps{c}")
            for c in range(GCHUNKS)
        ]
        for b in range(NBLK):
            rhs = psb.tile([128, SITE_EMB + 1], BF, tag="prhs")
            nc.sync.dma_start(rhs[:, 0:SITE_EMB], l2_slice[b * 128:(b + 1) * 128, :])
            nc.vector.tensor_copy(rhs[:, SITE_EMB:SITE_EMB + 1], ones_bf[:])
            gid = psb.tile([128, 1], F32, tag="gid")
            nc.sync.dma_start(gid[:], ins["gid_blk"][b, :, None])
            Sp = psb.tile([128, cfg.n_graphs], BF, tag="Spool")
            nc.vector.tensor_tensor(
                out=Sp[:], in0=gid[:, 0:1].to_broadcast([128, cfg.n_graphs]),
                in1=iota512[:], op=mybir.AluOpType.is_equal)
            for c in range(GCHUNKS):
                nc.tensor.matmul(
                    pool_ps[c][:], lhsT=Sp[:, c * 128:(c + 1) * 128], rhs=rhs[:],
                    start=(b == 0), stop=(b == NBLK - 1), skip_group_check=True)
        pstage = psb.tile([128, GCHUNKS, SITE_EMB + 1], F32, tag="pstage")
        for c in range(GCHUNKS):
            nc.vector.tensor_copy(pstage[:, c, :], pool_ps[c][:])
        # DRAM view: graph g = c*128 + p  ->  row-major [512, 65]
        nc.sync.dma_start(
            pool_part[:].rearrange("(c p) f -> p c f", p=128), pstage[:])

    if "P" in cfg.phases:
      nc.gpsimd.collective_compute(
        "AllReduce", mybir.AluOpType.add,
        replica_groups=[list(range(NC))],
        ins=[pool_part.opt()], outs=[pool_full.opt()],
      )

    # ---------------- Phase H: head MLP (replicated) ----------------
    if "H" in cfg.phases:
      with (
        tc.tile_pool(name="head_sb", bufs=1) as hsb,
        tc.tile_pool(name="head_ps", bufs=1, space="PSUM") as hps,
    ):
        pool_sb = hsb.tile([128, GCHUNKS, SITE_EMB + 1], F32)
        nc.sync.dma_start(
            pool_sb[:], pool_full[:].rearrange("(c p) f -> p c f", p=128))
        vecT = hsb.tile([SITE_EMB, GCHUNKS * 128], F32)
        for c in range(GCHUNKS):
            cnt = hsb.tile([128, 1], F32, tag="cnt")
            nc.vector.tensor_scalar_max(cnt[:], pool_sb[:, c, SITE_EMB:], 1.0)
            rec = hsb.tile([128, 1], F32, tag="rec")
            nc.vector.reciprocal(rec[:], cnt[:])
            vc = hsb.tile([128, SITE_EMB], F32, tag="vc")
            nc.vector.tensor_scalar_mul(vc[:], pool_sb[:, c, 0:SITE_EMB], rec[:, 0:1])
            vtp = hps.tile([SITE_EMB, 128], F32, tag="vtp", space="PSUM")
            nc.tensor.transpose(vtp[:], vc[:], identity[:])
            nc.vector.tensor_copy(vecT[:, c * 128:(c + 1) * 128], vtp[:])
        h1p = hps.tile([H1, cfg.n_graphs], F32, tag="h1p", space="PSUM")
        nc.tensor.matmul(h1p[:], lhsT=W1[:], rhs=vecT[:], start=True, stop=True)
        h1 = hsb.tile([H1, cfg.n_graphs], F32)
        nc.scalar.activation(h1[:], h1p[:], mybir.ActivationFunctionType.Relu,
                             bias=b1[:, 0:1])
        h2p = hps.tile([H2, cfg.n_graphs], F32, tag="h2p", space="PSUM")
        nc.tensor.matmul(h2p[:], lhsT=W2[:], rhs=h1[:], start=True, stop=True)
        h2 = hsb.tile([H2, cfg.n_graphs], F32)
        nc.scalar.activation(h2[:], h2p[:], mybir.ActivationFunctionType.Relu,
                             bias=b2[:, 0:1])
        op = hps.tile([OUT, cfg.n_graphs], F32, tag="op", space="PSUM")
        nc.tensor.matmul(op[:], lhsT=W3[:], rhs=h2[:], start=True, stop=True)
        ot = hsb.tile([OUT, cfg.n_graphs], F32)
        nc.vector.tensor_scalar_add(ot[:], op[:], b3[:, 0:1])
        nc.sync.dma_start(outs["out"][:].rearrange("g o -> o g"), ot[:])

    if getattr(cfg, "debug", False):
        for k in range(NC):
            nc.sync.dma_start(outs["dbg_tabA"][k * 128:(k + 1) * 128, :],
                              tabA[k * R:k * R + 128, :])
        nc.sync.dma_start(outs["dbg_l1"][:], l1_slice[:])
        nc.sync.dma_start(outs["dbg_tabB"][:], tabB[0:256, :])
        nc.sync.dma_start(outs["dbg_l2"][:], l2_slice[0:256, :])
        nc.sync.dma_start(outs["dbg_be"][:], beTab[0, :, :])
        nc.sync.dma_start(outs["dbg_pool"][:], pool_full[:])

    stack.close()


# ======================================================================
# Host-side preparation (pure data movement / index planning)
# ======================================================================

def prep_host(inputs, cfg):
    """Sort+pad edges, build per-core input maps. Returns list of dicts."""
    NC, NBLK, R, T0 = cfg.n_cores, cfg.nblk, cfg.R, cfg.T
    i1 = np.asarray(inputs["indices1"])
    i2 = np.asarray(inputs["indices2"])
    bonds = np.asarray(inputs["bonds"])
    n_sites = cfg.n_sites

    order = np.argsort(i1, kind="stable")
    i1s, i2s, bs = i1[order], i2[order], bonds[order]
    # per-core boundaries
    core_bounds = np.searchsorted(i1s, [c * R for c in range(NC + 1)])

    # per-(core, block) counts
    blk_of = (i1s // 128).astype(np.int64)  # global block id
    nblk_tot = NC * NBLK
    cnts = np.bincount(blk_of, minlength=nblk_tot)
    maxc = int(cnts.max()) if len(cnts) else 1
    T = max(2, int(np.ceil(maxc / 128.0)))
    T += T % 2
    if T0 is not None:
        assert T <= T0, f"data needs T={T} > configured {T0}"
        T = T0
    cfg.T = T

    cap = T * 128
    slot_arrays = {}
    # destination slot for each sorted edge: blk*cap + within-block index
    blk_starts = np.zeros(nblk_tot + 1, dtype=np.int64)
    np.cumsum(cnts, out=blk_starts[1:])
    within = np.arange(len(i1s), dtype=np.int64) - blk_starts[blk_of]
    slots = blk_of * cap + within

    def scatter(vals, fill, dtype):
        out = np.full(nblk_tot * cap, fill, dtype=dtype)
        out[slots] = vals.astype(dtype)
        return out.reshape(NC, NBLK, T, 128).transpose(0, 1, 3, 2).copy()

    blk_base = (np.arange(nblk_tot, dtype=np.int64) * 128)
    i1g = scatter(i1s, 0, np.int32)
    i2g = scatter(i2s, 0, np.int32)
    relv = i1s - blk_base[blk_of]
    rel = scatter(relv.astype(np.float32), 999.0, np.float32)
    bond_blk = scatter(bs.astype(np.float32), 0.0, np.float32)

    def quad_arrays(ig):
        # ig: [NC, NBLK, 128, T] int32 site indices (slot layout)
        qidx = (ig >> 2).astype(np.int16)
        sub = (ig & 3).astype(np.int32)
        # wrapped idx: flat j = t*128+p ; wrapped[p16, c] = q[c*16+p16], x8 replicated
        flat = qidx.transpose(0, 1, 3, 2).reshape(NC, NBLK, T * 128)
        wr = flat.reshape(NC, NBLK, T * 8, 16).transpose(0, 1, 3, 2)
        wrapped = np.tile(wr, (1, 1, 8, 1))  # [NC, NBLK, 128, T*8]
        masks = np.stack([(sub == k).astype(np.uint8) for k in (1, 2, 3)],
                         axis=3)  # [NC, NBLK, 128, 3, T]
        masks = masks.reshape(NC, NBLK, 128, 3 * T)
        return np.ascontiguousarray(wrapped), np.ascontiguousarray(masks)

    i1qw, i1mk = quad_arrays(i1g)
    i2qw, i2mk = quad_arrays(i2g)
    cfg._dbg_ig = (i1g, i2g)

    # site props slices (pad rows of zeros)
    sites = np.asarray(inputs["sites"], dtype=np.float32)
    sites_pad = np.zeros((cfg.site_pad, SITE_PROPS), dtype=np.float32)
    sites_pad[:n_sites] = sites
    g2s = np.asarray(inputs["graph_to_sites"])
    gid_pad = np.full(cfg.site_pad, 999.0, dtype=np.float32)
    gid_pad[:n_sites] = g2s.astype(np.float32)

    # constants
    centers = (np.arange(BOND_EXP, dtype=np.float32) * STEP)
    consts = {
        "identity128": np.eye(128, dtype=np.float32),
        "identity128_bf": np.eye(128).astype(BF16),
        "iota_row128": np.tile(np.arange(128, dtype=np.float32), (128, 1)),
        "iota512": np.tile(np.arange(cfg.n_graphs, dtype=np.float32), (128, 1)),
        "centers_row": np.tile(centers, (128, 1)),
        "ones_col": np.ones((128, 1), dtype=BF16),
        "Wse": np.asarray(inputs["W_se"], dtype=np.float32),
        "bse": np.asarray(inputs["b_se"], dtype=np.float32).reshape(SITE_EMB, 1),
        "Wbe_diag": np.block([
            [np.asarray(inputs["W_be"]), np.zeros((BOND_EXP, BOND_EMB))],
            [np.zeros((BOND_EXP, BOND_EMB)), np.asarray(inputs["W_be"])],
        ]).astype(BF16),
        "bbe2": np.concatenate([np.asarray(inputs["b_be"])] * 2).reshape(128, 1).astype(np.float32),
        "W1": np.asarray(inputs["W1"], dtype=np.float32),
        "b1": np.asarray(inputs["b1"], dtype=np.float32).reshape(H1, 1),
        "W2": np.asarray(inputs["W2"], dtype=np.float32),
        "b2": np.asarray(inputs["b2"], dtype=np.float32).reshape(H2, 1),
        "W3": np.asarray(inputs["W3"], dtype=np.float32),
        "b3": np.asarray(inputs["b3"], dtype=np.float32).reshape(1, 1),
    }
    gate_bias = False
    for L in (1, 2):
        for gate, wkey, bkey in ((
            "sig", f"W_sig{L}", f"b_sig{L}"), ("sof", f"W_sof{L}", f"b_sof{L}")):
            W = np.asarray(inputs[wkey], dtype=np.float32)  # [192, 64]
            consts[f"W{gate}{L}_top"] = W[0:128].astype(BF16)
            consts[f"W{gate}{L}_bot2"] = np.concatenate(
                [W[128:192], W[128:192]], axis=0).astype(BF16)
            bvec = np.asarray(inputs[bkey], dtype=np.float32).reshape(-1)
            if np.any(bvec != 0):
                gate_bias = True
            consts[f"b{gate}{L}_row"] = np.tile(
                np.tile(bvec, 8), (128, 1)).astype(np.float32)
    cfg.gate_bias = gate_bias
    if not gate_bias:
        for L in (1, 2):
            for gate in ("sig", "sof"):
                del consts[f"b{gate}{L}_row"]

    in_maps = []
    for c in range(NC):
        m = dict(consts)
        m["sites_slice"] = sites_pad[c * R:(c + 1) * R]
        m["gid_blk"] = gid_pad[c * R:(c + 1) * R].reshape(NBLK, 128)
        m["i1q_blk"] = i1qw[c]
        m["i2q_blk"] = i2qw[c]
        m["i1m_blk"] = i1mk[c]
        m["i2m_blk"] = i2mk[c]
        m["rel_blk"] = rel[c]
        m["bonds_blk"] = bond_blk[c]
        in_maps.append(m)
    return in_maps


def input_specs(cfg):
    NBLK, T, R = cfg.nblk, cfg.T, cfg.R
    specs = {}
    if cfg.gate_bias:
        for L in (1, 2):
            for gate in ("sig", "sof"):
                specs[f"b{gate}{L}_row"] = ([128, 8 * 64], F32)
    specs.update(_base_specs(cfg))
    return specs


def _base_specs(cfg):
    NBLK, T, R = cfg.nblk, cfg.T, cfg.R
    return {
        "sites_slice": ([R, SITE_PROPS], F32),
        "gid_blk": ([NBLK, 128], F32),
        "i1q_blk": ([NBLK, 128, T * 8], mybir.dt.int16),
        "i2q_blk": ([NBLK, 128, T * 8], mybir.dt.int16),
        "i1m_blk": ([NBLK, 128, 3 * T], mybir.dt.uint8),
        "i2m_blk": ([NBLK, 128, 3 * T], mybir.dt.uint8),
        "rel_blk": ([NBLK, 128, T], F32),
        "bonds_blk": ([NBLK, 128, T], F32),
        "identity128": ([128, 128], F32),
        "identity128_bf": ([128, 128], BF),
        "iota_row128": ([128, 128], F32),
        "iota512": ([128, cfg.n_graphs], F32),
        "centers_row": ([128, BOND_EXP], F32),
        "ones_col": ([128, 1], BF),
        "Wse": ([SITE_PROPS, SITE_EMB], F32),
        "bse": ([SITE_EMB, 1], F32),
        "Wbe_diag": ([128, 128], BF),
        "bbe2": ([128, 1], F32),
        "Wsig1_top": ([128, SITE_EMB], BF), "Wsig1_bot2": ([128, SITE_EMB], BF),
        "Wsof1_top": ([128, SITE_EMB], BF), "Wsof1_bot2": ([128, SITE_EMB], BF),
        "Wsig2_top": ([128, SITE_EMB], BF), "Wsig2_bot2": ([128, SITE_EMB], BF),
        "Wsof2_top": ([128, SITE_EMB], BF), "Wsof2_bot2": ([128, SITE_EMB], BF),
        "W1": ([SITE_EMB, H1], F32), "b1": ([H1, 1], F32),
        "W2": ([H1, H2], F32), "b2": ([H2, 1], F32),
        "W3": ([H2, OUT], F32), "b3": ([1, 1], F32),
    }


def build_bass(cfg):
    nc = bacc.Bacc("TRN2", target_bir_lowering=False, debug=False,
                   num_devices=cfg.n_cores)
    ins = {}
    for name, (shape, dt) in input_specs(cfg).items():
        ins[name] = nc.dram_tensor(name, shape, dt, kind="ExternalInput").ap()
    outs = {
        "out": nc.dram_tensor("out", [cfg.n_graphs, OUT], F32,
                              kind="ExternalOutput").ap()
    }
    if getattr(cfg, "debug", False):
        for nm, shape, dt in (
            ("dbg_tabA", [cfg.n_cores * 128, SITE_EMB], BF),
            ("dbg_l1", [cfg.R, SITE_EMB], BF),
            ("dbg_tabB", [256, SITE_EMB], BF),
            ("dbg_l2", [256, SITE_EMB], BF),
            ("dbg_be", [128, cfg.T * BOND_EMB], BF),
            ("dbg_pool", [cfg.n_graphs, SITE_EMB + 1], F32),
            ("dbg_s12", [128, cfg.T * 128], BF),
            ("dbg_s12T", [128, 128], BF),
            ("dbg_gmsg", [128, 512], BF),
        ):
            outs[nm] = nc.dram_tensor(nm, shape, dt, kind="ExternalOutput").ap()
    with tile.TileContext(nc) as tc:
        build_graph_kernel(nc, tc, ins, outs, cfg)
    nc.compile()
    return nc


_CACHE = {}


def run(inputs, cfg, **kw):
    in_maps = prep_host(inputs, cfg)
    key = (cfg.n_cores, cfg.nblk, cfg.T, cfg.site_pad, cfg.n_graphs,
           cfg.gate_bias)
    if key not in _CACHE:
        _CACHE[key] = build_bass(cfg)
    nc = _CACHE[key]
    res = run_bass_kernel_spmd(nc, in_maps, core_ids=list(range(cfg.n_cores)), **kw)
    return res


def kernel(**inputs) -> np.ndarray:
    n_sites = inputs["sites"].shape[0]
    cfg = Cfg(n_cores=8, nblk=98, T=None, n_sites=n_sites)
    res = run(inputs, cfg)
    return np.asarray(res.results[0]["out"], dtype=np.float32)


def build_calib(cfg):
    """Same inputs, trivial program — isolates launch+transfer overhead."""
    nc = bacc.Bacc("TRN2", target_bir_lowering=False, debug=False,
                   num_devices=cfg.n_cores)
    for name, (shape, dt) in input_specs(cfg).items():
        nc.dram_tensor(name, shape, dt, kind="ExternalInput").ap()
    out = nc.dram_tensor("out", [cfg.n_graphs, OUT], F32,
                         kind="ExternalOutput").ap()
    with tile.TileContext(nc) as tc:
        with tc.tile_pool(name="sb", bufs=1) as sb:
            t = sb.tile([1, cfg.n_graphs], F32)
            nc.vector.memset(t[:], 0.0)
            nc.sync.dma_start(out[:].rearrange("g o -> o g"), t[:])
    nc.compile()
    return nc



# revision 9
# speedup vs baseline: 1.0469x; 1.0469x over previous
import sys
import os

sys.path.insert(0, "/opt/trn_rl_repo")

import numpy as np
import ml_dtypes

import concourse.bass as bass
import concourse.mybir as mybir
import concourse.tile as tile
from concourse import bacc
from concourse.bass import IndirectOffsetOnAxis
from concourse.bass_utils import run_bass_kernel_spmd

BF16 = ml_dtypes.bfloat16

# model dims (fixed by the problem)
SITE_PROPS = 92
SITE_EMB = 64
BOND_EMB = 64
BOND_EXP = 64
MAX_DIST = 8.0
H1, H2, OUT = 128, 64, 1
N_GRAPHS = 512
GCHUNKS = N_GRAPHS // 128

F32 = mybir.dt.float32
BF = mybir.dt.bfloat16
I32 = mybir.dt.int32
I16 = mybir.dt.int16
U8 = mybir.dt.uint8

STEP = MAX_DIST / BOND_EXP
EXP_SCALE = -1.0 / (STEP * STEP)

G = 4  # tiles per activation group (PSUM bank budget)


class Cfg:
    def __init__(self, n_cores, nblk, T, n_sites, n_graphs=N_GRAPHS,
                 gate_bias=False):
        self.n_cores = n_cores
        self.nblk = nblk          # 128-site blocks per core
        self.T = T                # edge tiles (128 edges) per block, even
        self.R = nblk * 128       # sites per core
        self.site_pad = n_cores * self.R
        self.n_sites = n_sites
        self.n_graphs = n_graphs
        self.gate_bias = gate_bias
        self.phases = "EBC12PH"  # for bisection
        self.no_gather = False
        self.no_msg = False


def build_graph_kernel(nc, tc, ins, outs, cfg):
    """Build the full CGCNN forward pass. ins/outs: dicts of DRAM APs."""
    NBLK, T, R = cfg.nblk, cfg.T, cfg.R
    NC = cfg.n_cores
    PAIRS = T // 2
    groups = []
    t0 = 0
    while t0 < T:
        g = min(G, T - t0)
        groups.append((t0, g))
        t0 += g

    from contextlib import ExitStack
    stack = ExitStack()
    dram = stack.enter_context(tc.tile_pool(name="dram", bufs=1, space="DRAM"))
    tabA = dram.tile([cfg.site_pad, SITE_EMB], BF, addr_space="Shared")
    tabB = dram.tile([cfg.site_pad, SITE_EMB], BF, addr_space="Shared")
    emb_slice = dram.tile([R, SITE_EMB], BF)
    l1_slice = dram.tile([R, SITE_EMB], BF)
    l2_slice = dram.tile([R, SITE_EMB], BF)
    beTab = dram.tile([NBLK, 128, T * BOND_EMB], BF)     # transposed bexp
    pool_part = dram.tile([cfg.n_graphs, SITE_EMB + 1], F32)
    pool_full = dram.tile([cfg.n_graphs, SITE_EMB + 1], F32,
                          addr_space="Shared")

    cp = stack.enter_context(tc.tile_pool(name="consts", bufs=1))

    def load_const(name, shape, dtype):
        t = cp.tile(shape, dtype, name=f"c_{name}", tag=f"c_{name}")
        nc.sync.dma_start(t[:], ins[name][:])
        return t

    identity = load_const("identity128", [128, 128], F32)
    identity_bf = load_const("identity128_bf", [128, 128], BF)
    iota128 = load_const("iota_row128", [128, 128], F32)
    iota512 = load_const("iota512", [128, cfg.n_graphs], F32)
    centers = load_const("centers_row", [128, BOND_EXP], F32)
    ones_bf = load_const("ones_col", [128, 1], BF)
    Wse = load_const("Wse", [SITE_PROPS, SITE_EMB], F32)
    bse = load_const("bse", [SITE_EMB, 1], F32)
    Wa2 = {}
    Wb2 = {}
    Wc2 = {}
    for L in (1, 2):
        Wa2[L] = load_const(f"Wa2_{L}", [SITE_EMB, 128], BF)
        Wb2[L] = load_const(f"Wb2_{L}", [128, 128], BF)
        Wc2[L] = load_const(f"Wc2_{L}", [128, 128], BF)
    zbias = {}
    if cfg.gate_bias:
        for L in (1, 2):
            zbias[L] = load_const(f"zbias_{L}", [128, 128], F32)
    W1 = load_const("W1", [SITE_EMB, H1], F32)
    b1 = load_const("b1", [H1, 1], F32)
    W2 = load_const("W2", [H1, H2], F32)
    b2 = load_const("b2", [H2, 1], F32)
    W3 = load_const("W3", [H2, OUT], F32)
    b3 = load_const("b3", [1, 1], F32)

    # ---------------- Phase E: site embedding (own slice) ----------------
    if "E" in cfg.phases:
      with (
        tc.tile_pool(name="emb_sb", bufs=3) as esb,
        tc.tile_pool(name="emb_ps", bufs=2, space="PSUM") as eps,
      ):
        for b in range(NBLK):
            srow = esb.tile([128, SITE_PROPS], F32, tag="srow")
            nc.sync.dma_start(srow[:], ins["sites_slice"][b * 128:(b + 1) * 128, :])
            stp = eps.tile([SITE_PROPS, 128], F32, tag="stp", space="PSUM")
            nc.tensor.transpose(stp[:], srow[:], identity[:])
            sts = esb.tile([SITE_PROPS, 128], F32, tag="sts")
            nc.vector.tensor_copy(sts[:], stp[:])
            s0T = eps.tile([SITE_EMB, 128], F32, tag="s0T", space="PSUM")
            nc.tensor.matmul(s0T[:], lhsT=Wse[:], rhs=sts[:], start=True, stop=True)
            s0Tb = esb.tile([SITE_EMB, 128], F32, tag="s0Tb")
            nc.vector.tensor_scalar_add(s0Tb[:], s0T[:], bse[:, 0:1])
            s0p = eps.tile([128, SITE_EMB], F32, tag="s0p", space="PSUM")
            nc.tensor.transpose(s0p[:], s0Tb[:], identity[:SITE_EMB, :SITE_EMB])
            s0row = esb.tile([128, SITE_EMB], BF, tag="s0row")
            nc.vector.tensor_copy(s0row[:], s0p[:])
            nc.sync.dma_start(emb_slice[b * 128:(b + 1) * 128, :], s0row[:])

    if "E" in cfg.phases:
      nc.gpsimd.collective_compute(
        "AllGather", mybir.AluOpType.bypass,
        replica_groups=[list(range(NC))],
        ins=[emb_slice.opt()], outs=[tabA.opt()],
      )

    # ------- Phase B: transposed gaussian bond expansion (no matmul; -----
    # ------- the bond-embedding weights are folded into Wc2) -------------
    if "B" in cfg.phases:
      with (
        tc.tile_pool(name="be_sb", bufs=3) as bsb,
        tc.tile_pool(name="be_ps", bufs=2, space="PSUM") as bps,
      ):
        for b in range(NBLK):
            bonds = bsb.tile([128, T], F32, tag="bonds")
            nc.sync.dma_start(bonds[:], ins["bonds_blk"][b, :, :])
            d = bsb.tile([128, T, BOND_EXP], F32, tag="dmc")
            nc.vector.tensor_tensor(
                out=d[:], in0=bonds[:].to_broadcast([128, T, BOND_EXP]),
                in1=centers[:].to_broadcast([128, BOND_EXP, T]).rearrange(
                    "p c t -> p t c"),
                op=mybir.AluOpType.subtract)
            nc.vector.tensor_tensor(
                out=d[:], in0=d[:], in1=d[:], op=mybir.AluOpType.mult)
            bx = bsb.tile([128, T, BOND_EXP], BF, tag="bx")
            nc.scalar.activation(
                bx[:].rearrange("p t c -> p (t c)"),
                d[:].rearrange("p t c -> p (t c)"),
                mybir.ActivationFunctionType.Exp, scale=EXP_SCALE)
            stage = bsb.tile([128, T * BOND_EMB], BF, tag="beT_stage")
            for q in range(PAIRS):
                bxTp = bps.tile([128, 128], BF, tag="bexpTp", space="PSUM")
                nc.tensor.transpose(
                    bxTp[:],
                    bx[:, 2 * q:2 * q + 2, :].rearrange("p a b -> p (a b)"),
                    identity_bf[:])
                nc.vector.tensor_copy(stage[:, q * 128:(q + 1) * 128], bxTp[:])
            nc.sync.dma_start(beTab[b, :, :], stage[:])

    # ---------------- Phase C: conv layers ----------------
    def conv_layer(L, tab_in, res_in, slice_out):
        with (
            tc.tile_pool(name=f"c{L}_io", bufs=3) as iop,
            tc.tile_pool(name=f"c{L}_gq", bufs=3) as gtp,
            tc.tile_pool(name=f"c{L}_ms", bufs=3) as msb,
            tc.tile_pool(name=f"c{L}_zp", bufs=2, space="PSUM") as zps,
            tc.tile_pool(name=f"c{L}_ap", bufs=1, space="PSUM") as aps,
            tc.tile_pool(name=f"c{L}_tp", bufs=3, space="PSUM") as tps,
        ):
            tabq = tab_in[:].rearrange("(a b) c -> a (b c)", b=4)

            def load_block(b):
                d = {}
                i2q = iop.tile([128, T * 8], I16, tag="i2q")
                nc.sync.dma_start(i2q[:], ins["i2q_blk"][b, :, :])
                d["i2m"] = iop.tile([128, 3 * T], U8, tag="i2m", name="i2m")
                nc.sync.dma_start(d["i2m"][:], ins["i2m_blk"][b, :, :])
                d["rel"] = iop.tile([128, T], F32, tag="rel", name="rel")
                nc.sync.dma_start(d["rel"][:], ins["rel_blk"][b, :, :])
                d["beT"] = iop.tile([128, T * BOND_EMB], BF, tag="beT", name="beT")
                nc.sync.dma_start(d["beT"][:], beTab[b, :, :])
                d["own"] = iop.tile([128, SITE_EMB], BF, tag="own", name="own")
                nc.sync.dma_start(d["own"][:], res_in[b * 128:(b + 1) * 128, :])
                # one gather per tile-group, fanned across the 4 SWDGE
                # queues: descriptor generation is the bottleneck and it
                # parallelizes across queues
                d["q2"] = []
                for gi, (ts, gl) in enumerate(groups):
                    q2c = gtp.tile([128, G, 256], BF, tag=f"q2_{gi}", name=f"q2_{gi}")
                    if not cfg.no_gather:
                        nc.gpsimd.dma_gather(
                            q2c[:, 0:gl, :], tabq,
                            i2q[:, ts * 8:(ts + gl) * 8], gl * 128, gl * 128,
                            256, single_packet=False, queue_num=gi % 4)
                    d["q2"].append(q2c)
                return d

            def compute_block(b, d):
                own, q2, i2m = d["own"], d["q2"], d["i2m"]
                rel, beT = d["rel"], d["beT"]
                if cfg.no_msg:
                    snew = msb.tile([128, SITE_EMB], BF, tag="snew")
                    nc.vector.tensor_copy(snew[:], own[:])
                    nc.sync.dma_start(slice_out[b * 128:(b + 1) * 128, :],
                                      snew[:])
                    return
                # OWN_P = own @ [Wa_sig | Wa_sof]  (i1 side needs no gather:
                # edges are i1-sorted, so i1 always hits this block's rows)
                ownTp = tps.tile([128, 128], BF, tag="trB", space="PSUM")
                nc.tensor.transpose(ownTp[0:SITE_EMB, :], own[:], identity_bf[:])
                ownT = msb.tile([SITE_EMB, 128], BF, tag="ownT")
                nc.vector.tensor_copy(ownT[:], ownTp[0:SITE_EMB, :])
                OPp = aps.tile([128, 128], F32, tag="pf32", space="PSUM")
                nc.tensor.matmul(OPp[:], lhsT=ownT[:], rhs=Wa2[L][:],
                                 start=True, stop=True)
                OWN_P = msb.tile([128, 128], BF, tag="OWN_P")
                nc.vector.tensor_copy(OWN_P[:], OPp[:])

                aggT = aps.tile([SITE_EMB, 128], F32, tag="aggT", space="PSUM")
                for gi, (ts, gl) in enumerate(groups):
                    q2c = q2[gi]
                    s8g = msb.tile([128, G, 128], BF, tag="s8g")
                    nc.vector.tensor_tensor(
                        out=s8g[:, 0:gl, :],
                        in0=rel[:, ts:ts + gl].to_broadcast([128, gl, 128]),
                        in1=iota128[:].to_broadcast([128, 128, gl]).rearrange(
                            "p d g -> p g d"),
                        op=mybir.AluOpType.is_equal)
                    s2g = msb.tile([128, G, SITE_EMB], BF, tag="s2g")
                    dst = s2g[:, 0:gl, :]
                    nc.vector.tensor_copy(dst, q2c[:, 0:gl, 0:64])
                    for k in (1, 2, 3):
                        msk = i2m[:, (k - 1) * T + ts:(k - 1) * T + ts + gl]
                        nc.vector.copy_predicated(
                            dst, msk.to_broadcast([128, gl, 64]),
                            q2c[:, 0:gl, k * 64:(k + 1) * 64])
                    zg = zps.tile([128, G, 128], F32, tag="zg", space="PSUM")
                    s2T = {}
                    for ti in range(gl):
                        t = ts + ti
                        q, r = divmod(t, 2)
                        s8Tp = tps.tile([128, 128], BF, tag="trB",
                                        space="PSUM")
                        nc.tensor.transpose(s8Tp[:], s8g[:, ti, :],
                                            identity_bf[:])
                        s8T = msb.tile([128, 128], BF, tag="s8T")
                        nc.vector.tensor_copy(s8T[:], s8Tp[:])
                        if r == 0:
                            s2Tp = tps.tile([128, 128], BF, tag="trB",
                                            space="PSUM")
                            nc.tensor.transpose(
                                s2Tp[:],
                                s2g[:, ti:ti + 2, :].rearrange(
                                    "p a b -> p (a b)"),
                                identity_bf[:])
                            s2T[q] = msb.tile([128, 128], BF, tag="s2T", name="s2T")
                            nc.vector.tensor_copy(s2T[q][:], s2Tp[:])
                        nc.tensor.matmul(
                            zg[:, ti, :], lhsT=s8T[:], rhs=OWN_P[:],
                            start=True, stop=False)
                        nc.tensor.matmul(
                            zg[:, ti, :],
                            lhsT=s2T[q][64 * r:64 * (r + 1), :],
                            rhs=Wb2[L][64 * r:64 * (r + 1), :],
                            start=False, stop=False)
                        nc.tensor.matmul(
                            zg[:, ti, :],
                            lhsT=beT[64 * r:64 * (r + 1),
                                     q * 128:(q + 1) * 128],
                            rhs=Wc2[L][64 * r:64 * (r + 1), :],
                            start=False, stop=True)
                    if cfg.gate_bias:
                        nc.vector.tensor_tensor(
                            out=zg[:, 0:gl, :], in0=zg[:, 0:gl, :],
                            in1=zbias[L][:].to_broadcast(
                                [128, 128, gl]).rearrange("p f g -> p g f"),
                            op=mybir.AluOpType.add)
                    asig = msb.tile([128, G, 64], BF, tag="asig")
                    nc.scalar.activation(
                        asig[:, 0:gl, :], zg[:, 0:gl, 0:64],
                        mybir.ActivationFunctionType.Sigmoid)
                    asof = msb.tile([128, G, 64], BF, tag="asof")
                    nc.scalar.activation(
                        asof[:, 0:gl, :], zg[:, 0:gl, 64:128],
                        mybir.ActivationFunctionType.Relu)
                    gmsg = msb.tile([128, G, 64], BF, tag="gmsg")
                    nc.vector.tensor_tensor(
                        out=gmsg[:, 0:gl, :], in0=asig[:, 0:gl, :],
                        in1=asof[:, 0:gl, :], op=mybir.AluOpType.mult)
                    for ti in range(gl):
                        t = ts + ti
                        nc.tensor.matmul(
                            aggT[:], lhsT=gmsg[:, ti, :], rhs=s8g[:, ti, :],
                            start=(t == 0), stop=(t == T - 1),
                            skip_group_check=True)
                aggTs = msb.tile([SITE_EMB, 128], F32, tag="aggTs")
                nc.vector.tensor_copy(aggTs[:], aggT[:])
                aggp = aps.tile([128, 128], F32, tag="pf32", space="PSUM")
                nc.tensor.transpose(aggp[:, 0:SITE_EMB], aggTs[:],
                                    identity[:SITE_EMB, :SITE_EMB])
                snew = msb.tile([128, SITE_EMB], BF, tag="snew")
                nc.vector.tensor_tensor(
                    out=snew[:], in0=aggp[:, 0:SITE_EMB], in1=own[:],
                    op=mybir.AluOpType.add)
                nc.sync.dma_start(slice_out[b * 128:(b + 1) * 128, :], snew[:])

            cur = load_block(0)
            for b in range(NBLK):
                nxt = load_block(b + 1) if b + 1 < NBLK else None
                compute_block(b, cur)
                cur = nxt

    if "1" in cfg.phases:
        conv_layer(1, tabA, emb_slice, l1_slice)
        nc.gpsimd.collective_compute(
            "AllGather", mybir.AluOpType.bypass,
            replica_groups=[list(range(NC))],
            ins=[l1_slice.opt()], outs=[tabB.opt()],
        )
    if "2" in cfg.phases:
        conv_layer(2, tabB, l1_slice, l2_slice)

    # ---------------- Phase P: pooling over own sites ----------------
    if "P" in cfg.phases:
      with (
        tc.tile_pool(name="pool_sb", bufs=3) as psb,
        tc.tile_pool(name="pool_ps", bufs=1, space="PSUM") as pps,
      ):
        pool_ps = [
            pps.tile([128, SITE_EMB + 1], F32, tag=f"pool{c}", space="PSUM",
                     name=f"pool_ps{c}")
            for c in range(GCHUNKS)
        ]
        for b in range(NBLK):
            rhs = psb.tile([128, SITE_EMB + 1], BF, tag="prhs")
            nc.sync.dma_start(rhs[:, 0:SITE_EMB], l2_slice[b * 128:(b + 1) * 128, :])
            nc.vector.tensor_copy(rhs[:, SITE_EMB:SITE_EMB + 1], ones_bf[:])
            gid = psb.tile([128, 1], F32, tag="gid")
            nc.sync.dma_start(gid[:], ins["gid_blk"][b, :, None])
            Sp = psb.tile([128, cfg.n_graphs], BF, tag="Spool")
            nc.vector.tensor_tensor(
                out=Sp[:], in0=gid[:, 0:1].to_broadcast([128, cfg.n_graphs]),
                in1=iota512[:], op=mybir.AluOpType.is_equal)
            for c in range(GCHUNKS):
                nc.tensor.matmul(
                    pool_ps[c][:], lhsT=Sp[:, c * 128:(c + 1) * 128], rhs=rhs[:],
                    start=(b == 0), stop=(b == NBLK - 1), skip_group_check=True)
        pstage = psb.tile([128, GCHUNKS, SITE_EMB + 1], F32, tag="pstage")
        for c in range(GCHUNKS):
            nc.vector.tensor_copy(pstage[:, c, :], pool_ps[c][:])
        # DRAM view: graph g = c*128 + p  ->  row-major [512, 65]
        nc.sync.dma_start(
            pool_part[:].rearrange("(c p) f -> p c f", p=128), pstage[:])

    if "P" in cfg.phases:
      nc.gpsimd.collective_compute(
        "AllReduce", mybir.AluOpType.add,
        replica_groups=[list(range(NC))],
        ins=[pool_part.opt()], outs=[pool_full.opt()],
      )

    # ---------------- Phase H: head MLP (replicated) ----------------
    if "H" in cfg.phases:
      with (
        tc.tile_pool(name="head_sb", bufs=1) as hsb,
        tc.tile_pool(name="head_ps", bufs=1, space="PSUM") as hps,
      ):
        pool_sb = hsb.tile([128, GCHUNKS, SITE_EMB + 1], F32)
        nc.sync.dma_start(
            pool_sb[:], pool_full[:].rearrange("(c p) f -> p c f", p=128))
        vecT = hsb.tile([SITE_EMB, GCHUNKS * 128], F32)
        for c in range(GCHUNKS):
            cnt = hsb.tile([128, 1], F32, tag="cnt")
            nc.vector.tensor_scalar_max(cnt[:], pool_sb[:, c, SITE_EMB:], 1.0)
            rec = hsb.tile([128, 1], F32, tag="rec")
            nc.vector.reciprocal(rec[:], cnt[:])
            vc = hsb.tile([128, SITE_EMB], F32, tag="vc")
            nc.vector.tensor_scalar_mul(vc[:], pool_sb[:, c, 0:SITE_EMB], rec[:, 0:1])
            vtp = hps.tile([SITE_EMB, 128], F32, tag="vtp", space="PSUM")
            nc.tensor.transpose(vtp[:], vc[:], identity[:])
            nc.vector.tensor_copy(vecT[:, c * 128:(c + 1) * 128], vtp[:])
        h1p = hps.tile([H1, cfg.n_graphs], F32, tag="h1p", space="PSUM")
        nc.tensor.matmul(h1p[:], lhsT=W1[:], rhs=vecT[:], start=True, stop=True)
        h1 = hsb.tile([H1, cfg.n_graphs], F32)
        nc.scalar.activation(h1[:], h1p[:], mybir.ActivationFunctionType.Relu,
                             bias=b1[:, 0:1])
        h2p = hps.tile([H2, cfg.n_graphs], F32, tag="h2p", space="PSUM")
        nc.tensor.matmul(h2p[:], lhsT=W2[:], rhs=h1[:], start=True, stop=True)
        h2 = hsb.tile([H2, cfg.n_graphs], F32)
        nc.scalar.activation(h2[:], h2p[:], mybir.ActivationFunctionType.Relu,
                             bias=b2[:, 0:1])
        op = hps.tile([OUT, cfg.n_graphs], F32, tag="op", space="PSUM")
        nc.tensor.matmul(op[:], lhsT=W3[:], rhs=h2[:], start=True, stop=True)
        ot = hsb.tile([OUT, cfg.n_graphs], F32)
        nc.vector.tensor_scalar_add(ot[:], op[:], b3[:, 0:1])
        nc.sync.dma_start(outs["out"][:].rearrange("g o -> o g"), ot[:])

    stack.close()


# ======================================================================
# Host-side preparation (pure data movement / index planning)
# ======================================================================

def prep_host(inputs, cfg):
    """Sort+pad edges, build per-core input maps. Returns list of dicts."""
    NC, NBLK, R, T0 = cfg.n_cores, cfg.nblk, cfg.R, cfg.T
    i1 = np.asarray(inputs["indices1"])
    i2 = np.asarray(inputs["indices2"])
    bonds = np.asarray(inputs["bonds"])
    n_sites = cfg.n_sites

    order = np.argsort(i1, kind="stable")
    i1s, i2s, bs = i1[order], i2[order], bonds[order]

    # per-(core, block) counts
    blk_of = (i1s // 128).astype(np.int64)  # global block id
    nblk_tot = NC * NBLK
    cnts = np.bincount(blk_of, minlength=nblk_tot)
    maxc = int(cnts.max()) if len(cnts) else 1
    T = max(2, int(np.ceil(maxc / 128.0)))
    T += T % 2
    if T0 is not None:
        assert T <= T0, f"data needs T={T} > configured {T0}"
        T = T0
    cfg.T = T

    cap = T * 128
    # destination slot for each sorted edge: blk*cap + within-block index
    blk_starts = np.zeros(nblk_tot + 1, dtype=np.int64)
    np.cumsum(cnts, out=blk_starts[1:])
    within = np.arange(len(i1s), dtype=np.int64) - blk_starts[blk_of]
    slots = blk_of * cap + within

    def scatter(vals, fill, dtype):
        out = np.full(nblk_tot * cap, fill, dtype=dtype)
        out[slots] = vals.astype(dtype)
        return out.reshape(NC, NBLK, T, 128).transpose(0, 1, 3, 2).copy()

    blk_base = (np.arange(nblk_tot, dtype=np.int64) * 128)
    i2g = scatter(i2s, 0, np.int32)
    relv = i1s - blk_base[blk_of]
    rel = scatter(relv.astype(np.float32), 999.0, np.float32)
    bond_blk = scatter(bs.astype(np.float32), 0.0, np.float32)

    def quad_arrays(ig):
        # ig: [NC, NBLK, 128, T] int32 site indices (slot layout)
        qidx = (ig >> 2).astype(np.int16)
        sub = (ig & 3).astype(np.int32)
        # wrapped idx: flat j = t*128+p ; wrapped[p16, c] = q[c*16+p16], x8 replicated
        flat = qidx.transpose(0, 1, 3, 2).reshape(NC, NBLK, T * 128)
        wr = flat.reshape(NC, NBLK, T * 8, 16).transpose(0, 1, 3, 2)
        wrapped = np.tile(wr, (1, 1, 8, 1))  # [NC, NBLK, 128, T*8]
        masks = np.stack([(sub == k).astype(np.uint8) for k in (1, 2, 3)],
                         axis=3)  # [NC, NBLK, 128, 3, T]
        masks = masks.reshape(NC, NBLK, 128, 3 * T)
        return np.ascontiguousarray(wrapped), np.ascontiguousarray(masks)

    i2qw, i2mk = quad_arrays(i2g)

    # site props slices (pad rows of zeros)
    sites = np.asarray(inputs["sites"], dtype=np.float32)
    sites_pad = np.zeros((cfg.site_pad, SITE_PROPS), dtype=np.float32)
    sites_pad[:n_sites] = sites
    g2s = np.asarray(inputs["graph_to_sites"])
    gid_pad = np.full(cfg.site_pad, 999.0, dtype=np.float32)
    gid_pad[:n_sites] = g2s.astype(np.float32)

    # constants
    centers = (np.arange(BOND_EXP, dtype=np.float32) * STEP)
    consts = {
        "identity128": np.eye(128, dtype=np.float32),
        "identity128_bf": np.eye(128).astype(BF16),
        "iota_row128": np.tile(np.arange(128, dtype=np.float32), (128, 1)),
        "iota512": np.tile(np.arange(cfg.n_graphs, dtype=np.float32), (128, 1)),
        "centers_row": np.tile(centers, (128, 1)),
        "ones_col": np.ones((128, 1), dtype=BF16),
        "Wse": np.asarray(inputs["W_se"], dtype=np.float32),
        "bse": np.asarray(inputs["b_se"], dtype=np.float32).reshape(SITE_EMB, 1),
        "W1": np.asarray(inputs["W1"], dtype=np.float32),
        "b1": np.asarray(inputs["b1"], dtype=np.float32).reshape(H1, 1),
        "W2": np.asarray(inputs["W2"], dtype=np.float32),
        "b2": np.asarray(inputs["b2"], dtype=np.float32).reshape(H2, 1),
        "W3": np.asarray(inputs["W3"], dtype=np.float32),
        "b3": np.asarray(inputs["b3"], dtype=np.float32).reshape(1, 1),
    }
    Wbe = np.asarray(inputs["W_be"], dtype=np.float32)
    bbe = np.asarray(inputs["b_be"], dtype=np.float32).reshape(-1)
    gate_bias = False
    for L in (1, 2):
        Wsig = np.asarray(inputs[f"W_sig{L}"], dtype=np.float32)
        Wsof = np.asarray(inputs[f"W_sof{L}"], dtype=np.float32)
        bsig = np.asarray(inputs[f"b_sig{L}"], dtype=np.float32).reshape(-1)
        bsof = np.asarray(inputs[f"b_sof{L}"], dtype=np.float32).reshape(-1)
        consts[f"Wa2_{L}"] = np.concatenate(
            [Wsig[0:64], Wsof[0:64]], axis=1).astype(BF16)
        wb = np.concatenate([Wsig[64:128], Wsof[64:128]], axis=1)
        consts[f"Wb2_{L}"] = np.concatenate([wb, wb], axis=0).astype(BF16)
        wc = np.concatenate(
            [Wbe @ Wsig[128:192], Wbe @ Wsof[128:192]], axis=1)
        consts[f"Wc2_{L}"] = np.concatenate([wc, wc], axis=0).astype(BF16)
        zb = np.concatenate([bbe @ Wsig[128:192] + bsig,
                             bbe @ Wsof[128:192] + bsof])
        if np.any(zb != 0):
            gate_bias = True
        consts[f"zbias_{L}"] = np.tile(zb, (128, 1)).astype(np.float32)
    cfg.gate_bias = gate_bias
    if not gate_bias:
        for L in (1, 2):
            del consts[f"zbias_{L}"]

    in_maps = []
    for c in range(NC):
        m = dict(consts)
        m["sites_slice"] = sites_pad[c * R:(c + 1) * R]
        m["gid_blk"] = gid_pad[c * R:(c + 1) * R].reshape(NBLK, 128)
        m["i2q_blk"] = i2qw[c]
        m["i2m_blk"] = i2mk[c]
        m["rel_blk"] = rel[c]
        m["bonds_blk"] = bond_blk[c]
        in_maps.append(m)
    return in_maps


def input_specs(cfg):
    NBLK, T, R = cfg.nblk, cfg.T, cfg.R
    specs = {}
    if cfg.gate_bias:
        for L in (1, 2):
            specs[f"zbias_{L}"] = ([128, 128], F32)
    specs.update({
        "sites_slice": ([R, SITE_PROPS], F32),
        "gid_blk": ([NBLK, 128], F32),
        "i2q_blk": ([NBLK, 128, T * 8], I16),
        "i2m_blk": ([NBLK, 128, 3 * T], U8),
        "rel_blk": ([NBLK, 128, T], F32),
        "bonds_blk": ([NBLK, 128, T], F32),
        "identity128": ([128, 128], F32),
        "identity128_bf": ([128, 128], BF),
        "iota_row128": ([128, 128], F32),
        "iota512": ([128, cfg.n_graphs], F32),
        "centers_row": ([128, BOND_EXP], F32),
        "ones_col": ([128, 1], BF),
        "Wse": ([SITE_PROPS, SITE_EMB], F32),
        "bse": ([SITE_EMB, 1], F32),
        "Wa2_1": ([SITE_EMB, 128], BF), "Wb2_1": ([128, 128], BF),
        "Wc2_1": ([128, 128], BF),
        "Wa2_2": ([SITE_EMB, 128], BF), "Wb2_2": ([128, 128], BF),
        "Wc2_2": ([128, 128], BF),
        "W1": ([SITE_EMB, H1], F32), "b1": ([H1, 1], F32),
        "W2": ([H1, H2], F32), "b2": ([H2, 1], F32),
        "W3": ([H2, OUT], F32), "b3": ([1, 1], F32),
    })
    return specs


def build_bass(cfg):
    nc = bacc.Bacc("TRN2", target_bir_lowering=False, debug=False,
                   num_devices=cfg.n_cores, num_swdge_queues=4)
    ins = {}
    for name, (shape, dt) in input_specs(cfg).items():
        ins[name] = nc.dram_tensor(name, shape, dt, kind="ExternalInput").ap()
    outs = {
        "out": nc.dram_tensor("out", [cfg.n_graphs, OUT], F32,
                              kind="ExternalOutput").ap()
    }
    with tile.TileContext(nc) as tc:
        build_graph_kernel(nc, tc, ins, outs, cfg)
    nc.compile()
    return nc


_CACHE = {}


def run(inputs, cfg, **kw):
    in_maps = prep_host(inputs, cfg)
    key = (cfg.n_cores, cfg.nblk, cfg.T, cfg.site_pad, cfg.n_graphs,
           cfg.gate_bias, cfg.phases, cfg.no_gather, cfg.no_msg)
    if key not in _CACHE:
        _CACHE[key] = build_bass(cfg)
    nc = _CACHE[key]
    res = run_bass_kernel_spmd(nc, in_maps, core_ids=list(range(cfg.n_cores)), **kw)
    return res


def kernel(**inputs) -> np.ndarray:
    n_sites = inputs["sites"].shape[0]
    cfg = Cfg(n_cores=8, nblk=98, T=None, n_sites=n_sites)
    res = run(inputs, cfg)
    return np.asarray(res.results[0]["out"], dtype=np.float32)


def build_calib(cfg):
    """Same inputs, trivial program — isolates launch+transfer overhead."""
    nc = bacc.Bacc("TRN2", target_bir_lowering=False, debug=False,
                   num_devices=cfg.n_cores)
    for name, (shape, dt) in input_specs(cfg).items():
        nc.dram_tensor(name, shape, dt, kind="ExternalInput").ap()
    out = nc.dram_tensor("out", [cfg.n_graphs, OUT], F32,
                         kind="ExternalOutput").ap()
    with tile.TileContext(nc) as tc:
        with tc.tile_pool(name="sb", bufs=1) as sb:
            t = sb.tile([1, cfg.n_graphs], F32)
            nc.vector.memset(t[:], 0.0)
            nc.sync.dma_start(out[:].rearrange("g o -> o g"), t[:])
    nc.compile()
    return nc


# revision 12
# speedup vs baseline: 16.0917x; 15.3709x over previous
import sys
import os

sys.path.insert(0, "/opt/trn_rl_repo")

import numpy as np
import ml_dtypes

import concourse.bass as bass
import concourse.mybir as mybir
import concourse.tile as tile
from concourse import bacc
from concourse.bass import IndirectOffsetOnAxis
from concourse.bass_utils import run_bass_kernel_spmd

BF16 = ml_dtypes.bfloat16

# model dims (fixed by the problem)
SITE_PROPS = 92
SITE_EMB = 64
BOND_EMB = 64
BOND_EXP = 64
MAX_DIST = 8.0
H1, H2, OUT = 128, 64, 1
N_GRAPHS = 512
GCHUNKS = N_GRAPHS // 128

F32 = mybir.dt.float32
BF = mybir.dt.bfloat16
I32 = mybir.dt.int32
I16 = mybir.dt.int16
U8 = mybir.dt.uint8

STEP = MAX_DIST / BOND_EXP
EXP_SCALE = -1.0 / (STEP * STEP)

G = 4  # tiles per activation group (PSUM bank budget)


class Cfg:
    def __init__(self, n_cores, nblk, T, n_sites, n_graphs=N_GRAPHS,
                 gate_bias=False):
        self.n_cores = n_cores
        self.nblk = nblk          # 128-site blocks per core
        self.T = T                # edge tiles (128 edges) per block, even
        self.R = nblk * 128       # sites per core
        self.site_pad = n_cores * self.R
        self.n_sites = n_sites
        self.n_graphs = n_graphs
        self.gate_bias = gate_bias
        self.phases = "EBC12PH"  # for bisection
        self.no_gather = False
        self.no_msg = False


def build_graph_kernel(nc, tc, ins, outs, cfg):
    """Build the full CGCNN forward pass. ins/outs: dicts of DRAM APs."""
    NBLK, T, R = cfg.nblk, cfg.T, cfg.R
    NC = cfg.n_cores
    PAIRS = T // 2
    groups = []
    t0 = 0
    while t0 < T:
        g = min(G, T - t0)
        groups.append((t0, g))
        t0 += g

    from contextlib import ExitStack
    stack = ExitStack()
    dram = stack.enter_context(tc.tile_pool(name="dram", bufs=1, space="DRAM"))
    tabA = dram.tile([cfg.site_pad, SITE_EMB], BF, addr_space="Shared")
    tabB = dram.tile([cfg.site_pad, SITE_EMB], BF, addr_space="Shared")
    emb_slice = dram.tile([R, SITE_EMB], BF)
    l1_slice = dram.tile([R, SITE_EMB], BF)
    l2_slice = dram.tile([R, SITE_EMB], BF)
    beTab = dram.tile([NBLK, 128, T * BOND_EMB], BF)     # transposed bexp
    pool_part = dram.tile([cfg.n_graphs, SITE_EMB + 1], F32)
    pool_full = dram.tile([cfg.n_graphs, SITE_EMB + 1], F32,
                          addr_space="Shared")

    cp = stack.enter_context(tc.tile_pool(name="consts", bufs=1))

    def load_const(name, shape, dtype):
        t = cp.tile(shape, dtype, name=f"c_{name}", tag=f"c_{name}")
        nc.sync.dma_start(t[:], ins[name][:])
        return t

    identity = load_const("identity128", [128, 128], F32)
    identity_bf = load_const("identity128_bf", [128, 128], BF)
    iota128 = load_const("iota_row128", [128, 128], F32)
    iota512 = load_const("iota512", [128, cfg.n_graphs], F32)
    centers = load_const("centers_row", [128, BOND_EXP], F32)
    ones_bf = load_const("ones_col", [128, 1], BF)
    Wse = load_const("Wse", [SITE_PROPS, SITE_EMB], F32)
    bse = load_const("bse", [SITE_EMB, 1], F32)
    Wa2 = {}
    Wb2 = {}
    Wc2 = {}
    for L in (1, 2):
        Wa2[L] = load_const(f"Wa2_{L}", [SITE_EMB, 128], BF)
        Wb2[L] = load_const(f"Wb2_{L}", [128, 128], BF)
        Wc2[L] = load_const(f"Wc2_{L}", [128, 128], BF)
    zbias = {}
    if cfg.gate_bias:
        for L in (1, 2):
            zbias[L] = load_const(f"zbias_{L}", [128, 128], F32)
    W1 = load_const("W1", [SITE_EMB, H1], F32)
    b1 = load_const("b1", [H1, 1], F32)
    W2 = load_const("W2", [H1, H2], F32)
    b2 = load_const("b2", [H2, 1], F32)
    W3 = load_const("W3", [H2, OUT], F32)
    b3 = load_const("b3", [1, 1], F32)

    # ---------------- Phase E: site embedding (own slice) ----------------
    if "E" in cfg.phases:
      with (
        tc.tile_pool(name="emb_sb", bufs=3) as esb,
        tc.tile_pool(name="emb_ps", bufs=2, space="PSUM") as eps,
      ):
        for b in range(NBLK):
            srow = esb.tile([128, SITE_PROPS], F32, tag="srow")
            nc.sync.dma_start(srow[:], ins["sites_slice"][b * 128:(b + 1) * 128, :])
            stp = eps.tile([SITE_PROPS, 128], F32, tag="stp", space="PSUM")
            nc.tensor.transpose(stp[:], srow[:], identity[:])
            sts = esb.tile([SITE_PROPS, 128], F32, tag="sts")
            nc.vector.tensor_copy(sts[:], stp[:])
            s0T = eps.tile([SITE_EMB, 128], F32, tag="s0T", space="PSUM")
            nc.tensor.matmul(s0T[:], lhsT=Wse[:], rhs=sts[:], start=True, stop=True)
            s0Tb = esb.tile([SITE_EMB, 128], F32, tag="s0Tb")
            nc.vector.tensor_scalar_add(s0Tb[:], s0T[:], bse[:, 0:1])
            s0p = eps.tile([128, SITE_EMB], F32, tag="s0p", space="PSUM")
            nc.tensor.transpose(s0p[:], s0Tb[:], identity[:SITE_EMB, :SITE_EMB])
            s0row = esb.tile([128, SITE_EMB], BF, tag="s0row")
            nc.vector.tensor_copy(s0row[:], s0p[:])
            nc.sync.dma_start(emb_slice[b * 128:(b + 1) * 128, :], s0row[:])

    if "E" in cfg.phases:
      nc.gpsimd.collective_compute(
        "AllGather", mybir.AluOpType.bypass,
        replica_groups=[list(range(NC))],
        ins=[emb_slice.opt()], outs=[tabA.opt()],
      )

    # ------- Phase B: transposed gaussian bond expansion (no matmul; -----
    # ------- the bond-embedding weights are folded into Wc2) -------------
    if "B" in cfg.phases:
      with (
        tc.tile_pool(name="be_sb", bufs=3) as bsb,
        tc.tile_pool(name="be_ps", bufs=2, space="PSUM") as bps,
      ):
        for b in range(NBLK):
            bonds = bsb.tile([128, T], F32, tag="bonds")
            nc.sync.dma_start(bonds[:], ins["bonds_blk"][b, :, :])
            d = bsb.tile([128, T, BOND_EXP], F32, tag="dmc")
            nc.vector.tensor_tensor(
                out=d[:], in0=bonds[:].to_broadcast([128, T, BOND_EXP]),
                in1=centers[:].to_broadcast([128, BOND_EXP, T]).rearrange(
                    "p c t -> p t c"),
                op=mybir.AluOpType.subtract)
            nc.vector.tensor_tensor(
                out=d[:], in0=d[:], in1=d[:], op=mybir.AluOpType.mult)
            bx = bsb.tile([128, T, BOND_EXP], BF, tag="bx")
            nc.scalar.activation(
                bx[:].rearrange("p t c -> p (t c)"),
                d[:].rearrange("p t c -> p (t c)"),
                mybir.ActivationFunctionType.Exp, scale=EXP_SCALE)
            stage = bsb.tile([128, T * BOND_EMB], BF, tag="beT_stage")
            for q in range(PAIRS):
                bxTp = bps.tile([128, 128], BF, tag="bexpTp", space="PSUM")
                nc.tensor.transpose(
                    bxTp[:],
                    bx[:, 2 * q:2 * q + 2, :].rearrange("p a b -> p (a b)"),
                    identity_bf[:])
                nc.vector.tensor_copy(stage[:, q * 128:(q + 1) * 128], bxTp[:])
            nc.sync.dma_start(beTab[b, :, :], stage[:])

    # ---------------- Phase C: conv layers ----------------
    def conv_layer(L, tab_in, res_in, slice_out):
        with (
            tc.tile_pool(name=f"c{L}_io", bufs=4) as iop,
            tc.tile_pool(name=f"c{L}_gq", bufs=4) as gtp,
            tc.tile_pool(name=f"c{L}_ms", bufs=3) as msb,
            tc.tile_pool(name=f"c{L}_zp", bufs=2, space="PSUM") as zps,
            tc.tile_pool(name=f"c{L}_ap", bufs=1, space="PSUM") as aps,
            tc.tile_pool(name=f"c{L}_tp", bufs=3, space="PSUM") as tps,
        ):
            tabq = tab_in[:].rearrange("(a b) c -> a (b c)", b=4)

            def load_block(b):
                d = {}
                i2q = iop.tile([128, T * 8], I16, tag="i2q")
                nc.sync.dma_start(i2q[:], ins["i2q_blk"][b, :, :])
                d["i2m"] = iop.tile([128, 3 * T], U8, tag="i2m", name="i2m")
                nc.sync.dma_start(d["i2m"][:], ins["i2m_blk"][b, :, :])
                d["rel"] = iop.tile([128, T], F32, tag="rel", name="rel")
                nc.sync.dma_start(d["rel"][:], ins["rel_blk"][b, :, :])
                d["beT"] = iop.tile([128, T * BOND_EMB], BF, tag="beT", name="beT")
                nc.sync.dma_start(d["beT"][:], beTab[b, :, :])
                d["own"] = iop.tile([128, SITE_EMB], BF, tag="own", name="own")
                nc.sync.dma_start(d["own"][:], res_in[b * 128:(b + 1) * 128, :])
                # one gather per tile-group, fanned across the 4 SWDGE
                # queues: descriptor generation is the bottleneck and it
                # parallelizes across queues
                d["q2"] = []
                for gi, (ts, gl) in enumerate(groups):
                    q2c = gtp.tile([128, G, 256], BF, tag=f"q2_{gi}", name=f"q2_{gi}")
                    if not cfg.no_gather:
                        nc.gpsimd.dma_gather(
                            q2c[:, 0:gl, :], tabq,
                            i2q[:, ts * 8:(ts + gl) * 8], gl * 128, gl * 128,
                            256, single_packet=False, queue_num=gi % 4)
                    d["q2"].append(q2c)
                return d

            def compute_block(b, d):
                own, q2, i2m = d["own"], d["q2"], d["i2m"]
                rel, beT = d["rel"], d["beT"]
                if cfg.no_msg:
                    snew = msb.tile([128, SITE_EMB], BF, tag="snew")
                    nc.vector.tensor_copy(snew[:], own[:])
                    nc.sync.dma_start(slice_out[b * 128:(b + 1) * 128, :],
                                      snew[:])
                    return
                # OWN_P = own @ [Wa_sig | Wa_sof]  (i1 side needs no gather:
                # edges are i1-sorted, so i1 always hits this block's rows)
                ownTp = tps.tile([128, 128], BF, tag="trB", space="PSUM")
                nc.tensor.transpose(ownTp[0:SITE_EMB, :], own[:], identity_bf[:])
                ownT = msb.tile([SITE_EMB, 128], BF, tag="ownT")
                nc.vector.tensor_copy(ownT[:], ownTp[0:SITE_EMB, :])
                OPp = aps.tile([128, 128], F32, tag="pf32", space="PSUM")
                nc.tensor.matmul(OPp[:], lhsT=ownT[:], rhs=Wa2[L][:],
                                 start=True, stop=True)
                OWN_P = msb.tile([128, 128], BF, tag="OWN_P")
                nc.vector.tensor_copy(OWN_P[:], OPp[:])

                aggT = aps.tile([SITE_EMB, 128], F32, tag="aggT", space="PSUM")
                for gi, (ts, gl) in enumerate(groups):
                    q2c = q2[gi]
                    s8g = msb.tile([128, G, 128], BF, tag="s8g")
                    nc.vector.tensor_tensor(
                        out=s8g[:, 0:gl, :],
                        in0=rel[:, ts:ts + gl].to_broadcast([128, gl, 128]),
                        in1=iota128[:].to_broadcast([128, 128, gl]).rearrange(
                            "p d g -> p g d"),
                        op=mybir.AluOpType.is_equal)
                    s2g = msb.tile([128, G, SITE_EMB], BF, tag="s2g")
                    dst = s2g[:, 0:gl, :]
                    nc.vector.tensor_copy(dst, q2c[:, 0:gl, 0:64])
                    for k in (1, 2, 3):
                        msk = i2m[:, (k - 1) * T + ts:(k - 1) * T + ts + gl]
                        nc.vector.copy_predicated(
                            dst, msk.to_broadcast([128, gl, 64]),
                            q2c[:, 0:gl, k * 64:(k + 1) * 64])
                    zg = zps.tile([128, G, 128], F32, tag="zg", space="PSUM")
                    s2T = {}
                    for ti in range(gl):
                        t = ts + ti
                        q, r = divmod(t, 2)
                        s8Tp = tps.tile([128, 128], BF, tag="trB",
                                        space="PSUM")
                        nc.tensor.transpose(s8Tp[:], s8g[:, ti, :],
                                            identity_bf[:])
                        s8T = msb.tile([128, 128], BF, tag="s8T")
                        nc.vector.tensor_copy(s8T[:], s8Tp[:])
                        if r == 0:
                            s2Tp = tps.tile([128, 128], BF, tag="trB",
                                            space="PSUM")
                            nc.tensor.transpose(
                                s2Tp[:],
                                s2g[:, ti:ti + 2, :].rearrange(
                                    "p a b -> p (a b)"),
                                identity_bf[:])
                            s2T[q] = msb.tile([128, 128], BF, tag="s2T", name="s2T")
                            nc.vector.tensor_copy(s2T[q][:], s2Tp[:])
                        nc.tensor.matmul(
                            zg[:, ti, :], lhsT=s8T[:], rhs=OWN_P[:],
                            start=True, stop=False)
                        nc.tensor.matmul(
                            zg[:, ti, :],
                            lhsT=s2T[q][64 * r:64 * (r + 1), :],
                            rhs=Wb2[L][64 * r:64 * (r + 1), :],
                            start=False, stop=False)
                        nc.tensor.matmul(
                            zg[:, ti, :],
                            lhsT=beT[64 * r:64 * (r + 1),
                                     q * 128:(q + 1) * 128],
                            rhs=Wc2[L][64 * r:64 * (r + 1), :],
                            start=False, stop=True)
                    if cfg.gate_bias:
                        nc.vector.tensor_tensor(
                            out=zg[:, 0:gl, :], in0=zg[:, 0:gl, :],
                            in1=zbias[L][:].to_broadcast(
                                [128, 128, gl]).rearrange("p f g -> p g f"),
                            op=mybir.AluOpType.add)
                    asig = msb.tile([128, G, 64], BF, tag="asig")
                    nc.scalar.activation(
                        asig[:, 0:gl, :], zg[:, 0:gl, 0:64],
                        mybir.ActivationFunctionType.Sigmoid)
                    asof = msb.tile([128, G, 64], BF, tag="asof")
                    nc.scalar.activation(
                        asof[:, 0:gl, :], zg[:, 0:gl, 64:128],
                        mybir.ActivationFunctionType.Relu)
                    gmsg = msb.tile([128, G, 64], BF, tag="gmsg")
                    nc.vector.tensor_tensor(
                        out=gmsg[:, 0:gl, :], in0=asig[:, 0:gl, :],
                        in1=asof[:, 0:gl, :], op=mybir.AluOpType.mult)
                    for ti in range(gl):
                        t = ts + ti
                        nc.tensor.matmul(
                            aggT[:], lhsT=gmsg[:, ti, :], rhs=s8g[:, ti, :],
                            start=(t == 0), stop=(t == T - 1),
                            skip_group_check=True)
                aggTs = msb.tile([SITE_EMB, 128], F32, tag="aggTs")
                nc.vector.tensor_copy(aggTs[:], aggT[:])
                aggp = aps.tile([128, 128], F32, tag="pf32", space="PSUM")
                nc.tensor.transpose(aggp[:, 0:SITE_EMB], aggTs[:],
                                    identity[:SITE_EMB, :SITE_EMB])
                snew = msb.tile([128, SITE_EMB], BF, tag="snew")
                nc.vector.tensor_tensor(
                    out=snew[:], in0=aggp[:, 0:SITE_EMB], in1=own[:],
                    op=mybir.AluOpType.add)
                nc.sync.dma_start(slice_out[b * 128:(b + 1) * 128, :], snew[:])

            pending = [load_block(0), load_block(1)]
            for b in range(NBLK):
                if b + 2 < NBLK:
                    pending.append(load_block(b + 2))
                compute_block(b, pending.pop(0))

    if "1" in cfg.phases:
        conv_layer(1, tabA, emb_slice, l1_slice)
        nc.gpsimd.collective_compute(
            "AllGather", mybir.AluOpType.bypass,
            replica_groups=[list(range(NC))],
            ins=[l1_slice.opt()], outs=[tabB.opt()],
        )
    if "2" in cfg.phases:
        conv_layer(2, tabB, l1_slice, l2_slice)

    # ---------------- Phase P: pooling over own sites ----------------
    if "P" in cfg.phases:
      with (
        tc.tile_pool(name="pool_sb", bufs=3) as psb,
        tc.tile_pool(name="pool_ps", bufs=1, space="PSUM") as pps,
      ):
        pool_ps = [
            pps.tile([128, SITE_EMB + 1], F32, tag=f"pool{c}", space="PSUM",
                     name=f"pool_ps{c}")
            for c in range(GCHUNKS)
        ]
        for b in range(NBLK):
            rhs = psb.tile([128, SITE_EMB + 1], BF, tag="prhs")
            nc.sync.dma_start(rhs[:, 0:SITE_EMB], l2_slice[b * 128:(b + 1) * 128, :])
            nc.vector.tensor_copy(rhs[:, SITE_EMB:SITE_EMB + 1], ones_bf[:])
            gid = psb.tile([128, 1], F32, tag="gid")
            nc.sync.dma_start(gid[:], ins["gid_blk"][b, :, None])
            Sp = psb.tile([128, cfg.n_graphs], BF, tag="Spool")
            nc.vector.tensor_tensor(
                out=Sp[:], in0=gid[:, 0:1].to_broadcast([128, cfg.n_graphs]),
                in1=iota512[:], op=mybir.AluOpType.is_equal)
            for c in range(GCHUNKS):
                nc.tensor.matmul(
                    pool_ps[c][:], lhsT=Sp[:, c * 128:(c + 1) * 128], rhs=rhs[:],
                    start=(b == 0), stop=(b == NBLK - 1), skip_group_check=True)
        pstage = psb.tile([128, GCHUNKS, SITE_EMB + 1], F32, tag="pstage")
        for c in range(GCHUNKS):
            nc.vector.tensor_copy(pstage[:, c, :], pool_ps[c][:])
        # DRAM view: graph g = c*128 + p  ->  row-major [512, 65]
        nc.sync.dma_start(
            pool_part[:].rearrange("(c p) f -> p c f", p=128), pstage[:])

    if "P" in cfg.phases:
      nc.gpsimd.collective_compute(
        "AllReduce", mybir.AluOpType.add,
        replica_groups=[list(range(NC))],
        ins=[pool_part.opt()], outs=[pool_full.opt()],
      )

    # ---------------- Phase H: head MLP (replicated) ----------------
    if "H" in cfg.phases:
      with (
        tc.tile_pool(name="head_sb", bufs=1) as hsb,
        tc.tile_pool(name="head_ps", bufs=1, space="PSUM") as hps,
      ):
        pool_sb = hsb.tile([128, GCHUNKS, SITE_EMB + 1], F32)
        nc.sync.dma_start(
            pool_sb[:], pool_full[:].rearrange("(c p) f -> p c f", p=128))
        vecT = hsb.tile([SITE_EMB, GCHUNKS * 128], F32)
        for c in range(GCHUNKS):
            cnt = hsb.tile([128, 1], F32, tag="cnt")
            nc.vector.tensor_scalar_max(cnt[:], pool_sb[:, c, SITE_EMB:], 1.0)
            rec = hsb.tile([128, 1], F32, tag="rec")
            nc.vector.reciprocal(rec[:], cnt[:])
            vc = hsb.tile([128, SITE_EMB], F32, tag="vc")
            nc.vector.tensor_scalar_mul(vc[:], pool_sb[:, c, 0:SITE_EMB], rec[:, 0:1])
            vtp = hps.tile([SITE_EMB, 128], F32, tag="vtp", space="PSUM")
            nc.tensor.transpose(vtp[:], vc[:], identity[:])
            nc.vector.tensor_copy(vecT[:, c * 128:(c + 1) * 128], vtp[:])
        h1p = hps.tile([H1, cfg.n_graphs], F32, tag="h1p", space="PSUM")
        nc.tensor.matmul(h1p[:], lhsT=W1[:], rhs=vecT[:], start=True, stop=True)
        h1 = hsb.tile([H1, cfg.n_graphs], F32)
        nc.scalar.activation(h1[:], h1p[:], mybir.ActivationFunctionType.Relu,
                             bias=b1[:, 0:1])
        h2p = hps.tile([H2, cfg.n_graphs], F32, tag="h2p", space="PSUM")
        nc.tensor.matmul(h2p[:], lhsT=W2[:], rhs=h1[:], start=True, stop=True)
        h2 = hsb.tile([H2, cfg.n_graphs], F32)
        nc.scalar.activation(h2[:], h2p[:], mybir.ActivationFunctionType.Relu,
                             bias=b2[:, 0:1])
        op = hps.tile([OUT, cfg.n_graphs], F32, tag="op", space="PSUM")
        nc.tensor.matmul(op[:], lhsT=W3[:], rhs=h2[:], start=True, stop=True)
        ot = hsb.tile([OUT, cfg.n_graphs], F32)
        nc.vector.tensor_scalar_add(ot[:], op[:], b3[:, 0:1])
        nc.sync.dma_start(outs["out"][:].rearrange("g o -> o g"), ot[:])

    stack.close()


# ======================================================================
# Host-side preparation (pure data movement / index planning)
# ======================================================================

def prep_host(inputs, cfg):
    """Sort+pad edges, build per-core input maps. Returns list of dicts."""
    NC, NBLK, R, T0 = cfg.n_cores, cfg.nblk, cfg.R, cfg.T
    i1 = np.asarray(inputs["indices1"])
    i2 = np.asarray(inputs["indices2"])
    bonds = np.asarray(inputs["bonds"])
    n_sites = cfg.n_sites

    order = np.argsort(i1, kind="stable")
    i1s, i2s, bs = i1[order], i2[order], bonds[order]

    # per-(core, block) counts
    blk_of = (i1s // 128).astype(np.int64)  # global block id
    nblk_tot = NC * NBLK
    cnts = np.bincount(blk_of, minlength=nblk_tot)
    maxc = int(cnts.max()) if len(cnts) else 1
    T = max(2, int(np.ceil(maxc / 128.0)))
    T += T % 2
    if T0 is not None:
        assert T <= T0, f"data needs T={T} > configured {T0}"
        T = T0
    cfg.T = T

    cap = T * 128
    # destination slot for each sorted edge: blk*cap + within-block index
    blk_starts = np.zeros(nblk_tot + 1, dtype=np.int64)
    np.cumsum(cnts, out=blk_starts[1:])
    within = np.arange(len(i1s), dtype=np.int64) - blk_starts[blk_of]
    slots = blk_of * cap + within

    def scatter(vals, fill, dtype):
        out = np.full(nblk_tot * cap, fill, dtype=dtype)
        out[slots] = vals.astype(dtype)
        return out.reshape(NC, NBLK, T, 128).transpose(0, 1, 3, 2).copy()

    blk_base = (np.arange(nblk_tot, dtype=np.int64) * 128)
    i2g = scatter(i2s, 0, np.int32)
    relv = i1s - blk_base[blk_of]
    rel = scatter(relv.astype(np.float32), 999.0, np.float32)
    bond_blk = scatter(bs.astype(np.float32), 0.0, np.float32)

    def quad_arrays(ig):
        # ig: [NC, NBLK, 128, T] int32 site indices (slot layout)
        qidx = (ig >> 2).astype(np.int16)
        sub = (ig & 3).astype(np.int32)
        # wrapped idx: flat j = t*128+p ; wrapped[p16, c] = q[c*16+p16], x8 replicated
        flat = qidx.transpose(0, 1, 3, 2).reshape(NC, NBLK, T * 128)
        wr = flat.reshape(NC, NBLK, T * 8, 16).transpose(0, 1, 3, 2)
        wrapped = np.tile(wr, (1, 1, 8, 1))  # [NC, NBLK, 128, T*8]
        masks = np.stack([(sub == k).astype(np.uint8) for k in (1, 2, 3)],
                         axis=3)  # [NC, NBLK, 128, 3, T]
        masks = masks.reshape(NC, NBLK, 128, 3 * T)
        return np.ascontiguousarray(wrapped), np.ascontiguousarray(masks)

    i2qw, i2mk = quad_arrays(i2g)

    # site props slices (pad rows of zeros)
    sites = np.asarray(inputs["sites"], dtype=np.float32)
    sites_pad = np.zeros((cfg.site_pad, SITE_PROPS), dtype=np.float32)
    sites_pad[:n_sites] = sites
    g2s = np.asarray(inputs["graph_to_sites"])
    gid_pad = np.full(cfg.site_pad, 999.0, dtype=np.float32)
    gid_pad[:n_sites] = g2s.astype(np.float32)

    # constants
    centers = (np.arange(BOND_EXP, dtype=np.float32) * STEP)
    consts = {
        "identity128": np.eye(128, dtype=np.float32),
        "identity128_bf": np.eye(128).astype(BF16),
        "iota_row128": np.tile(np.arange(128, dtype=np.float32), (128, 1)),
        "iota512": np.tile(np.arange(cfg.n_graphs, dtype=np.float32), (128, 1)),
        "centers_row": np.tile(centers, (128, 1)),
        "ones_col": np.ones((128, 1), dtype=BF16),
        "Wse": np.asarray(inputs["W_se"], dtype=np.float32),
        "bse": np.asarray(inputs["b_se"], dtype=np.float32).reshape(SITE_EMB, 1),
        "W1": np.asarray(inputs["W1"], dtype=np.float32),
        "b1": np.asarray(inputs["b1"], dtype=np.float32).reshape(H1, 1),
        "W2": np.asarray(inputs["W2"], dtype=np.float32),
        "b2": np.asarray(inputs["b2"], dtype=np.float32).reshape(H2, 1),
        "W3": np.asarray(inputs["W3"], dtype=np.float32),
        "b3": np.asarray(inputs["b3"], dtype=np.float32).reshape(1, 1),
    }
    Wbe = np.asarray(inputs["W_be"], dtype=np.float32)
    bbe = np.asarray(inputs["b_be"], dtype=np.float32).reshape(-1)
    gate_bias = False
    for L in (1, 2):
        Wsig = np.asarray(inputs[f"W_sig{L}"], dtype=np.float32)
        Wsof = np.asarray(inputs[f"W_sof{L}"], dtype=np.float32)
        bsig = np.asarray(inputs[f"b_sig{L}"], dtype=np.float32).reshape(-1)
        bsof = np.asarray(inputs[f"b_sof{L}"], dtype=np.float32).reshape(-1)
        consts[f"Wa2_{L}"] = np.concatenate(
            [Wsig[0:64], Wsof[0:64]], axis=1).astype(BF16)
        wb = np.concatenate([Wsig[64:128], Wsof[64:128]], axis=1)
        consts[f"Wb2_{L}"] = np.concatenate([wb, wb], axis=0).astype(BF16)
        wc = np.concatenate(
            [Wbe @ Wsig[128:192], Wbe @ Wsof[128:192]], axis=1)
        consts[f"Wc2_{L}"] = np.concatenate([wc, wc], axis=0).astype(BF16)
        zb = np.concatenate([bbe @ Wsig[128:192] + bsig,
                             bbe @ Wsof[128:192] + bsof])
        if np.any(zb != 0):
            gate_bias = True
        consts[f"zbias_{L}"] = np.tile(zb, (128, 1)).astype(np.float32)
    cfg.gate_bias = gate_bias
    if not gate_bias:
        for L in (1, 2):
            del consts[f"zbias_{L}"]

    in_maps = []
    for c in range(NC):
        m = dict(consts)
        m["sites_slice"] = sites_pad[c * R:(c + 1) * R]
        m["gid_blk"] = gid_pad[c * R:(c + 1) * R].reshape(NBLK, 128)
        m["i2q_blk"] = i2qw[c]
        m["i2m_blk"] = i2mk[c]
        m["rel_blk"] = rel[c]
        m["bonds_blk"] = bond_blk[c]
        in_maps.append(m)
    return in_maps


def input_specs(cfg):
    NBLK, T, R = cfg.nblk, cfg.T, cfg.R
    specs = {}
    if cfg.gate_bias:
        for L in (1, 2):
            specs[f"zbias_{L}"] = ([128, 128], F32)
    specs.update({
        "sites_slice": ([R, SITE_PROPS], F32),
        "gid_blk": ([NBLK, 128], F32),
        "i2q_blk": ([NBLK, 128, T * 8], I16),
        "i2m_blk": ([NBLK, 128, 3 * T], U8),
        "rel_blk": ([NBLK, 128, T], F32),
        "bonds_blk": ([NBLK, 128, T], F32),
        "identity128": ([128, 128], F32),
        "identity128_bf": ([128, 128], BF),
        "iota_row128": ([128, 128], F32),
        "iota512": ([128, cfg.n_graphs], F32),
        "centers_row": ([128, BOND_EXP], F32),
        "ones_col": ([128, 1], BF),
        "Wse": ([SITE_PROPS, SITE_EMB], F32),
        "bse": ([SITE_EMB, 1], F32),
        "Wa2_1": ([SITE_EMB, 128], BF), "Wb2_1": ([128, 128], BF),
        "Wc2_1": ([128, 128], BF),
        "Wa2_2": ([SITE_EMB, 128], BF), "Wb2_2": ([128, 128], BF),
        "Wc2_2": ([128, 128], BF),
        "W1": ([SITE_EMB, H1], F32), "b1": ([H1, 1], F32),
        "W2": ([H1, H2], F32), "b2": ([H2, 1], F32),
        "W3": ([H2, OUT], F32), "b3": ([1, 1], F32),
    })
    return specs


def build_bass(cfg):
    nc = bacc.Bacc("TRN2", target_bir_lowering=False, debug=False,
                   num_devices=cfg.n_cores, num_swdge_queues=4)
    ins = {}
    for name, (shape, dt) in input_specs(cfg).items():
        ins[name] = nc.dram_tensor(name, shape, dt, kind="ExternalInput").ap()
    outs = {
        "out": nc.dram_tensor("out", [cfg.n_graphs, OUT], F32,
                              kind="ExternalOutput").ap()
    }
    with tile.TileContext(nc) as tc:
        build_graph_kernel(nc, tc, ins, outs, cfg)
    nc.compile()
    return nc


_CACHE = {}


def run(inputs, cfg, **kw):
    in_maps = prep_host(inputs, cfg)
    key = (cfg.n_cores, cfg.nblk, cfg.T, cfg.site_pad, cfg.n_graphs,
           cfg.gate_bias, cfg.phases, cfg.no_gather, cfg.no_msg)
    if key not in _CACHE:
        _CACHE[key] = build_bass(cfg)
    nc = _CACHE[key]
    res = run_bass_kernel_spmd(nc, in_maps, core_ids=list(range(cfg.n_cores)), **kw)
    return res


def kernel(**inputs) -> np.ndarray:
    n_sites = inputs["sites"].shape[0]
    cfg = Cfg(n_cores=8, nblk=98, T=None, n_sites=n_sites)
    res = run(inputs, cfg)
    return np.asarray(res.results[0]["out"], dtype=np.float32)


def build_calib(cfg):
    """Same inputs, trivial program — isolates launch+transfer overhead."""
    nc = bacc.Bacc("TRN2", target_bir_lowering=False, debug=False,
                   num_devices=cfg.n_cores)
    for name, (shape, dt) in input_specs(cfg).items():
        nc.dram_tensor(name, shape, dt, kind="ExternalInput").ap()
    out = nc.dram_tensor("out", [cfg.n_graphs, OUT], F32,
                         kind="ExternalOutput").ap()
    with tile.TileContext(nc) as tc:
        with tc.tile_pool(name="sb", bufs=1) as sb:
            t = sb.tile([1, cfg.n_graphs], F32)
            nc.vector.memset(t[:], 0.0)
            nc.sync.dma_start(out[:].rearrange("g o -> o g"), t[:])
    nc.compile()
    return nc


# revision 16
# speedup vs baseline: 18.0695x; 1.1229x over previous
import sys
import os

sys.path.insert(0, "/opt/trn_rl_repo")

import numpy as np
import ml_dtypes

import concourse.bass as bass
import concourse.mybir as mybir
import concourse.tile as tile
from concourse import bacc
from concourse.bass import IndirectOffsetOnAxis
from concourse.bass_utils import run_bass_kernel_spmd

BF16 = ml_dtypes.bfloat16

# model dims (fixed by the problem)
SITE_PROPS = 92
SITE_EMB = 64
BOND_EMB = 64
BOND_EXP = 64
MAX_DIST = 8.0
H1, H2, OUT = 128, 64, 1
N_GRAPHS = 512
GCHUNKS = N_GRAPHS // 128

F32 = mybir.dt.float32
BF = mybir.dt.bfloat16
I32 = mybir.dt.int32
I16 = mybir.dt.int16
U8 = mybir.dt.uint8

STEP = MAX_DIST / BOND_EXP
EXP_SCALE = -1.0 / (STEP * STEP)

G = 4  # tiles per activation group (PSUM bank budget)


class Cfg:
    def __init__(self, n_cores, nblk, T, n_sites, n_graphs=N_GRAPHS,
                 gate_bias=False):
        self.n_cores = n_cores
        self.nblk = nblk          # 128-site blocks per core
        self.T = T                # edge tiles (128 edges) per block, even
        self.R = nblk * 128       # sites per core
        self.site_pad = n_cores * self.R
        self.n_sites = n_sites
        self.n_graphs = n_graphs
        self.gate_bias = gate_bias
        self.phases = "EBC12PH"  # for bisection
        self.no_gather = False
        self.no_msg = False


def build_graph_kernel(nc, tc, ins, outs, cfg):
    """Build the full CGCNN forward pass. ins/outs: dicts of DRAM APs."""
    NBLK, T, R = cfg.nblk, cfg.T, cfg.R
    NC = cfg.n_cores
    PAIRS = T // 2
    groups = []
    t0 = 0
    while t0 < T:
        g = min(G, T - t0)
        groups.append((t0, g))
        t0 += g

    from contextlib import ExitStack
    stack = ExitStack()
    dram = stack.enter_context(tc.tile_pool(name="dram", bufs=1, space="DRAM"))
    tabA = dram.tile([cfg.site_pad, SITE_EMB], BF, addr_space="Shared")
    tabB = dram.tile([cfg.site_pad, SITE_EMB], BF, addr_space="Shared")
    emb_slice = dram.tile([R, SITE_EMB], BF)
    l1_slice = dram.tile([R, SITE_EMB], BF)
    l2_slice = dram.tile([R, SITE_EMB], BF)
    beTab = dram.tile([NBLK, 128, T * BOND_EMB], BF)     # transposed bexp
    pool_part = dram.tile([cfg.n_graphs, SITE_EMB + 1], F32)
    pool_full = dram.tile([cfg.n_graphs, SITE_EMB + 1], F32,
                          addr_space="Shared")

    cp = stack.enter_context(tc.tile_pool(name="consts", bufs=1))

    def load_const(name, shape, dtype):
        t = cp.tile(shape, dtype, name=f"c_{name}", tag=f"c_{name}")
        nc.sync.dma_start(t[:], ins[name][:])
        return t

    identity = load_const("identity128", [128, 128], F32)
    identity_bf = load_const("identity128_bf", [128, 128], BF)
    iota128 = load_const("iota_row128", [128, 128], F32)
    iota512 = load_const("iota512", [128, cfg.n_graphs], F32)
    centers = load_const("centers_row", [128, BOND_EXP], F32)
    ones_bf = load_const("ones_col", [128, 1], BF)
    Wse = load_const("Wse", [SITE_PROPS, SITE_EMB], F32)
    bse = load_const("bse", [SITE_EMB, 1], F32)
    Wa2 = {}
    Wb2 = {}
    Wc2 = {}
    for L in (1, 2):
        Wa2[L] = load_const(f"Wa2_{L}", [SITE_EMB, 128], BF)
        Wb2[L] = load_const(f"Wb2_{L}", [128, 128], BF)
        Wc2[L] = load_const(f"Wc2_{L}", [128, 128], BF)
    zbias = {}
    if cfg.gate_bias:
        for L in (1, 2):
            zbias[L] = load_const(f"zbias_{L}", [128, 128], F32)
    W1 = load_const("W1", [SITE_EMB, H1], F32)
    b1 = load_const("b1", [H1, 1], F32)
    W2 = load_const("W2", [H1, H2], F32)
    b2 = load_const("b2", [H2, 1], F32)
    W3 = load_const("W3", [H2, OUT], F32)
    b3 = load_const("b3", [1, 1], F32)

    # ---------------- Phase E: site embedding (own slice) ----------------
    if "E" in cfg.phases:
      with (
        tc.tile_pool(name="emb_sb", bufs=3) as esb,
        tc.tile_pool(name="emb_ps", bufs=2, space="PSUM") as eps,
      ):
        for b in range(NBLK):
            srow = esb.tile([128, SITE_PROPS], F32, tag="srow")
            nc.sync.dma_start(srow[:], ins["sites_slice"][b * 128:(b + 1) * 128, :])
            stp = eps.tile([SITE_PROPS, 128], F32, tag="stp", space="PSUM")
            nc.tensor.transpose(stp[:], srow[:], identity[:])
            sts = esb.tile([SITE_PROPS, 128], F32, tag="sts")
            nc.vector.tensor_copy(sts[:], stp[:])
            s0T = eps.tile([SITE_EMB, 128], F32, tag="s0T", space="PSUM")
            nc.tensor.matmul(s0T[:], lhsT=Wse[:], rhs=sts[:], start=True, stop=True)
            s0Tb = esb.tile([SITE_EMB, 128], F32, tag="s0Tb")
            nc.vector.tensor_scalar_add(s0Tb[:], s0T[:], bse[:, 0:1])
            s0p = eps.tile([128, SITE_EMB], F32, tag="s0p", space="PSUM")
            nc.tensor.transpose(s0p[:], s0Tb[:], identity[:SITE_EMB, :SITE_EMB])
            s0row = esb.tile([128, SITE_EMB], BF, tag="s0row")
            nc.vector.tensor_copy(s0row[:], s0p[:])
            nc.sync.dma_start(emb_slice[b * 128:(b + 1) * 128, :], s0row[:])

    if "E" in cfg.phases:
      nc.gpsimd.collective_compute(
        "AllGather", mybir.AluOpType.bypass,
        replica_groups=[list(range(NC))],
        ins=[emb_slice.opt()], outs=[tabA.opt()],
      )

    # ------- Phase B: transposed gaussian bond expansion (no matmul; -----
    # ------- the bond-embedding weights are folded into Wc2) -------------
    if "B" in cfg.phases:
      with (
        tc.tile_pool(name="be_sb", bufs=3) as bsb,
        tc.tile_pool(name="be_ps", bufs=2, space="PSUM") as bps,
      ):
        for b in range(NBLK):
            bonds = bsb.tile([128, T], F32, tag="bonds")
            nc.sync.dma_start(bonds[:], ins["bonds_blk"][b, :, :])
            d = bsb.tile([128, T, BOND_EXP], F32, tag="dmc")
            nc.vector.tensor_tensor(
                out=d[:], in0=bonds[:].to_broadcast([128, T, BOND_EXP]),
                in1=centers[:].to_broadcast([128, BOND_EXP, T]).rearrange(
                    "p c t -> p t c"),
                op=mybir.AluOpType.subtract)
            nc.vector.tensor_tensor(
                out=d[:], in0=d[:], in1=d[:], op=mybir.AluOpType.mult)
            bx = bsb.tile([128, T, BOND_EXP], BF, tag="bx")
            nc.scalar.activation(
                bx[:].rearrange("p t c -> p (t c)"),
                d[:].rearrange("p t c -> p (t c)"),
                mybir.ActivationFunctionType.Exp, scale=EXP_SCALE)
            stage = bsb.tile([128, T * BOND_EMB], BF, tag="beT_stage")
            for q in range(PAIRS):
                bxTp = bps.tile([128, 128], BF, tag="bexpTp", space="PSUM")
                nc.tensor.transpose(
                    bxTp[:],
                    bx[:, 2 * q:2 * q + 2, :].rearrange("p a b -> p (a b)"),
                    identity_bf[:])
                nc.vector.tensor_copy(stage[:, q * 128:(q + 1) * 128], bxTp[:])
            nc.sync.dma_start(beTab[b, :, :], stage[:])

    # ---------------- Phase C: conv layers ----------------
    def conv_layer(L, tab_in, res_in, slice_out):
        with (
            tc.tile_pool(name=f"c{L}_io", bufs=4) as iop,
            tc.tile_pool(name=f"c{L}_gq", bufs=4) as gtp,
            tc.tile_pool(name=f"c{L}_ms", bufs=3) as msb,
            tc.tile_pool(name=f"c{L}_zp", bufs=2, space="PSUM") as zps,
            tc.tile_pool(name=f"c{L}_ap", bufs=1, space="PSUM") as aps,
            tc.tile_pool(name=f"c{L}_tp", bufs=3, space="PSUM") as tps,
        ):
            tabq = tab_in[:].rearrange("(a b) c -> a (b c)", b=4)

            def load_block(b):
                d = {}
                BPAD = (-23 * T) % 4
                blob = iop.tile([128, 23 * T + BPAD], U8, tag="blob",
                                name="blob")
                nc.sync.dma_start(blob[:], ins["pack_blk"][b, :, :])
                d["rel"] = blob[:, 0:4 * T].bitcast(F32)
                i2q = blob[:, 4 * T:20 * T].bitcast(I16)
                d["i2m"] = blob[:, 20 * T:23 * T]
                d["beT"] = iop.tile([128, T * BOND_EMB], BF, tag="beT", name="beT")
                nc.scalar.dma_start(d["beT"][:], beTab[b, :, :])
                d["own"] = iop.tile([128, SITE_EMB], BF, tag="own", name="own")
                nc.scalar.dma_start(d["own"][:], res_in[b * 128:(b + 1) * 128, :])
                # one gather per tile-group, fanned across the 4 SWDGE
                # queues: descriptor generation is the bottleneck and it
                # parallelizes across queues
                d["q2"] = []
                for gi, (ts, gl) in enumerate(groups):
                    q2c = gtp.tile([128, G, 256], BF, tag=f"q2_{gi}", name=f"q2_{gi}")
                    if not cfg.no_gather:
                        nc.gpsimd.dma_gather(
                            q2c[:, 0:gl, :], tabq,
                            i2q[:, ts * 8:(ts + gl) * 8], gl * 128, gl * 128,
                            256, single_packet=False, queue_num=gi % 4)
                    d["q2"].append(q2c)
                return d

            def compute_block(b, d):
                own, q2, i2m = d["own"], d["q2"], d["i2m"]
                rel, beT = d["rel"], d["beT"]
                if cfg.no_msg:
                    snew = msb.tile([128, SITE_EMB], BF, tag="snew")
                    nc.vector.tensor_copy(snew[:], own[:])
                    nc.sync.dma_start(slice_out[b * 128:(b + 1) * 128, :],
                                      snew[:])
                    return
                # OWN_P = own @ [Wa_sig | Wa_sof]  (i1 side needs no gather:
                # edges are i1-sorted, so i1 always hits this block's rows)
                ownTp = tps.tile([128, 128], BF, tag="trB", space="PSUM")
                nc.tensor.transpose(ownTp[0:SITE_EMB, :], own[:], identity_bf[:])
                ownT = msb.tile([SITE_EMB, 128], BF, tag="ownT")
                nc.vector.tensor_copy(ownT[:], ownTp[0:SITE_EMB, :])
                OPp = aps.tile([128, 128], F32, tag="pf32", space="PSUM")
                nc.tensor.matmul(OPp[:], lhsT=ownT[:], rhs=Wa2[L][:],
                                 start=True, stop=True)
                OWN_P = msb.tile([128, 128], BF, tag="OWN_P")
                nc.vector.tensor_copy(OWN_P[:], OPp[:])

                aggT = aps.tile([SITE_EMB, 128], F32, tag="aggT", space="PSUM")
                for gi, (ts, gl) in enumerate(groups):
                    q2c = q2[gi]
                    s8g = msb.tile([128, G, 128], BF, tag="s8g")
                    nc.vector.tensor_tensor(
                        out=s8g[:, 0:gl, :],
                        in0=rel[:, ts:ts + gl].to_broadcast([128, gl, 128]),
                        in1=iota128[:].to_broadcast([128, 128, gl]).rearrange(
                            "p d g -> p g d"),
                        op=mybir.AluOpType.is_equal)
                    s2g = msb.tile([128, G, SITE_EMB], BF, tag="s2g")
                    dst = s2g[:, 0:gl, :]
                    nc.vector.tensor_copy(dst, q2c[:, 0:gl, 0:64])
                    for k in (1, 2, 3):
                        msk = i2m[:, (k - 1) * T + ts:(k - 1) * T + ts + gl]
                        nc.vector.copy_predicated(
                            dst, msk.to_broadcast([128, gl, 64]),
                            q2c[:, 0:gl, k * 64:(k + 1) * 64])
                    zg = zps.tile([128, G, 128], F32, tag="zg", space="PSUM")
                    s2T = {}
                    for ti in range(gl):
                        t = ts + ti
                        q, r = divmod(t, 2)
                        s8Tp = tps.tile([128, 128], BF, tag="trB",
                                        space="PSUM")
                        nc.tensor.transpose(s8Tp[:], s8g[:, ti, :],
                                            identity_bf[:])
                        s8T = msb.tile([128, 128], BF, tag="s8T")
                        nc.vector.tensor_copy(s8T[:], s8Tp[:])
                        if r == 0:
                            s2Tp = tps.tile([128, 128], BF, tag="trB",
                                            space="PSUM")
                            nc.tensor.transpose(
                                s2Tp[:],
                                s2g[:, ti:ti + 2, :].rearrange(
                                    "p a b -> p (a b)"),
                                identity_bf[:])
                            s2T[q] = msb.tile([128, 128], BF, tag="s2T", name="s2T")
                            nc.vector.tensor_copy(s2T[q][:], s2Tp[:])
                        nc.tensor.matmul(
                            zg[:, ti, :], lhsT=s8T[:], rhs=OWN_P[:],
                            start=True, stop=False)
                        nc.tensor.matmul(
                            zg[:, ti, :],
                            lhsT=s2T[q][64 * r:64 * (r + 1), :],
                            rhs=Wb2[L][64 * r:64 * (r + 1), :],
                            start=False, stop=False)
                        nc.tensor.matmul(
                            zg[:, ti, :],
                            lhsT=beT[64 * r:64 * (r + 1),
                                     q * 128:(q + 1) * 128],
                            rhs=Wc2[L][64 * r:64 * (r + 1), :],
                            start=False, stop=True)
                    if cfg.gate_bias:
                        nc.vector.tensor_tensor(
                            out=zg[:, 0:gl, :], in0=zg[:, 0:gl, :],
                            in1=zbias[L][:].to_broadcast(
                                [128, 128, gl]).rearrange("p f g -> p g f"),
                            op=mybir.AluOpType.add)
                    asig = msb.tile([128, G, 64], BF, tag="asig")
                    nc.scalar.activation(
                        asig[:, 0:gl, :], zg[:, 0:gl, 0:64],
                        mybir.ActivationFunctionType.Sigmoid)
                    asof = msb.tile([128, G, 64], BF, tag="asof")
                    nc.scalar.activation(
                        asof[:, 0:gl, :], zg[:, 0:gl, 64:128],
                        mybir.ActivationFunctionType.Relu)
                    gmsg = msb.tile([128, G, 64], BF, tag="gmsg")
                    nc.vector.tensor_tensor(
                        out=gmsg[:, 0:gl, :], in0=asig[:, 0:gl, :],
                        in1=asof[:, 0:gl, :], op=mybir.AluOpType.mult)
                    for ti in range(gl):
                        t = ts + ti
                        nc.tensor.matmul(
                            aggT[:], lhsT=gmsg[:, ti, :], rhs=s8g[:, ti, :],
                            start=(t == 0), stop=(t == T - 1),
                            skip_group_check=True)
                aggTs = msb.tile([SITE_EMB, 128], F32, tag="aggTs")
                nc.vector.tensor_copy(aggTs[:], aggT[:])
                aggp = aps.tile([128, 128], F32, tag="pf32", space="PSUM")
                nc.tensor.transpose(aggp[:, 0:SITE_EMB], aggTs[:],
                                    identity[:SITE_EMB, :SITE_EMB])
                snew = msb.tile([128, SITE_EMB], BF, tag="snew")
                nc.vector.tensor_tensor(
                    out=snew[:], in0=aggp[:, 0:SITE_EMB], in1=own[:],
                    op=mybir.AluOpType.add)
                nc.sync.dma_start(slice_out[b * 128:(b + 1) * 128, :], snew[:])

            pending = [load_block(0), load_block(1)]
            for b in range(NBLK):
                if b + 2 < NBLK:
                    pending.append(load_block(b + 2))
                compute_block(b, pending.pop(0))

    if "1" in cfg.phases:
        conv_layer(1, tabA, emb_slice, l1_slice)
        nc.gpsimd.collective_compute(
            "AllGather", mybir.AluOpType.bypass,
            replica_groups=[list(range(NC))],
            ins=[l1_slice.opt()], outs=[tabB.opt()],
        )
    if "2" in cfg.phases:
        conv_layer(2, tabB, l1_slice, l2_slice)

    # ---------------- Phase P: pooling over own sites ----------------
    if "P" in cfg.phases:
      with (
        tc.tile_pool(name="pool_sb", bufs=3) as psb,
        tc.tile_pool(name="pool_ps", bufs=1, space="PSUM") as pps,
      ):
        pool_ps = [
            pps.tile([128, SITE_EMB + 1], F32, tag=f"pool{c}", space="PSUM",
                     name=f"pool_ps{c}")
            for c in range(GCHUNKS)
        ]
        for b in range(NBLK):
            rhs = psb.tile([128, SITE_EMB + 1], BF, tag="prhs")
            nc.sync.dma_start(rhs[:, 0:SITE_EMB], l2_slice[b * 128:(b + 1) * 128, :])
            nc.vector.tensor_copy(rhs[:, SITE_EMB:SITE_EMB + 1], ones_bf[:])
            gid = psb.tile([128, 1], F32, tag="gid")
            nc.sync.dma_start(gid[:], ins["gid_blk"][b, :, None])
            Sp = psb.tile([128, cfg.n_graphs], BF, tag="Spool")
            nc.vector.tensor_tensor(
                out=Sp[:], in0=gid[:, 0:1].to_broadcast([128, cfg.n_graphs]),
                in1=iota512[:], op=mybir.AluOpType.is_equal)
            for c in range(GCHUNKS):
                nc.tensor.matmul(
                    pool_ps[c][:], lhsT=Sp[:, c * 128:(c + 1) * 128], rhs=rhs[:],
                    start=(b == 0), stop=(b == NBLK - 1), skip_group_check=True)
        pstage = psb.tile([128, GCHUNKS, SITE_EMB + 1], F32, tag="pstage")
        for c in range(GCHUNKS):
            nc.vector.tensor_copy(pstage[:, c, :], pool_ps[c][:])
        # DRAM view: graph g = c*128 + p  ->  row-major [512, 65]
        nc.sync.dma_start(
            pool_part[:].rearrange("(c p) f -> p c f", p=128), pstage[:])

    if "P" in cfg.phases:
      nc.gpsimd.collective_compute(
        "AllReduce", mybir.AluOpType.add,
        replica_groups=[list(range(NC))],
        ins=[pool_part.opt()], outs=[pool_full.opt()],
      )

    # ---------------- Phase H: head MLP (replicated) ----------------
    if "H" in cfg.phases:
      with (
        tc.tile_pool(name="head_sb", bufs=1) as hsb,
        tc.tile_pool(name="head_ps", bufs=1, space="PSUM") as hps,
      ):
        pool_sb = hsb.tile([128, GCHUNKS, SITE_EMB + 1], F32)
        nc.sync.dma_start(
            pool_sb[:], pool_full[:].rearrange("(c p) f -> p c f", p=128))
        vecT = hsb.tile([SITE_EMB, GCHUNKS * 128], F32)
        for c in range(GCHUNKS):
            cnt = hsb.tile([128, 1], F32, tag="cnt")
            nc.vector.tensor_scalar_max(cnt[:], pool_sb[:, c, SITE_EMB:], 1.0)
            rec = hsb.tile([128, 1], F32, tag="rec")
            nc.vector.reciprocal(rec[:], cnt[:])
            vc = hsb.tile([128, SITE_EMB], F32, tag="vc")
            nc.vector.tensor_scalar_mul(vc[:], pool_sb[:, c, 0:SITE_EMB], rec[:, 0:1])
            vtp = hps.tile([SITE_EMB, 128], F32, tag="vtp", space="PSUM")
            nc.tensor.transpose(vtp[:], vc[:], identity[:])
            nc.vector.tensor_copy(vecT[:, c * 128:(c + 1) * 128], vtp[:])
        h1p = hps.tile([H1, cfg.n_graphs], F32, tag="h1p", space="PSUM")
        nc.tensor.matmul(h1p[:], lhsT=W1[:], rhs=vecT[:], start=True, stop=True)
        h1 = hsb.tile([H1, cfg.n_graphs], F32)
        nc.scalar.activation(h1[:], h1p[:], mybir.ActivationFunctionType.Relu,
                             bias=b1[:, 0:1])
        h2p = hps.tile([H2, cfg.n_graphs], F32, tag="h2p", space="PSUM")
        nc.tensor.matmul(h2p[:], lhsT=W2[:], rhs=h1[:], start=True, stop=True)
        h2 = hsb.tile([H2, cfg.n_graphs], F32)
        nc.scalar.activation(h2[:], h2p[:], mybir.ActivationFunctionType.Relu,
                             bias=b2[:, 0:1])
        op = hps.tile([OUT, cfg.n_graphs], F32, tag="op", space="PSUM")
        nc.tensor.matmul(op[:], lhsT=W3[:], rhs=h2[:], start=True, stop=True)
        ot = hsb.tile([OUT, cfg.n_graphs], F32)
        nc.vector.tensor_scalar_add(ot[:], op[:], b3[:, 0:1])
        nc.sync.dma_start(outs["out"][:].rearrange("g o -> o g"), ot[:])

    stack.close()


# ======================================================================
# Host-side preparation (pure data movement / index planning)
# ======================================================================

def prep_host(inputs, cfg):
    """Sort+pad edges, build per-core input maps. Returns list of dicts."""
    NC, NBLK, R, T0 = cfg.n_cores, cfg.nblk, cfg.R, cfg.T
    i1 = np.asarray(inputs["indices1"])
    i2 = np.asarray(inputs["indices2"])
    bonds = np.asarray(inputs["bonds"])
    n_sites = cfg.n_sites

    order = np.argsort(i1, kind="stable")
    i1s, i2s, bs = i1[order], i2[order], bonds[order]

    # per-(core, block) counts
    blk_of = (i1s // 128).astype(np.int64)  # global block id
    nblk_tot = NC * NBLK
    cnts = np.bincount(blk_of, minlength=nblk_tot)
    maxc = int(cnts.max()) if len(cnts) else 1
    T = max(2, int(np.ceil(maxc / 128.0)))
    T += T % 2
    if T0 is not None:
        assert T <= T0, f"data needs T={T} > configured {T0}"
        T = T0
    cfg.T = T

    cap = T * 128
    # destination slot for each sorted edge: blk*cap + within-block index
    blk_starts = np.zeros(nblk_tot + 1, dtype=np.int64)
    np.cumsum(cnts, out=blk_starts[1:])
    within = np.arange(len(i1s), dtype=np.int64) - blk_starts[blk_of]
    slots = blk_of * cap + within

    def scatter(vals, fill, dtype):
        out = np.full(nblk_tot * cap, fill, dtype=dtype)
        out[slots] = vals.astype(dtype)
        return out.reshape(NC, NBLK, T, 128).transpose(0, 1, 3, 2).copy()

    blk_base = (np.arange(nblk_tot, dtype=np.int64) * 128)
    i2g = scatter(i2s, 0, np.int32)
    relv = i1s - blk_base[blk_of]
    rel = scatter(relv.astype(np.float32), 999.0, np.float32)
    bond_blk = scatter(bs.astype(np.float32), 0.0, np.float32)

    def quad_arrays(ig):
        # ig: [NC, NBLK, 128, T] int32 site indices (slot layout)
        qidx = (ig >> 2).astype(np.int16)
        sub = (ig & 3).astype(np.int32)
        # wrapped idx: flat j = t*128+p ; wrapped[p16, c] = q[c*16+p16], x8 replicated
        flat = qidx.transpose(0, 1, 3, 2).reshape(NC, NBLK, T * 128)
        wr = flat.reshape(NC, NBLK, T * 8, 16).transpose(0, 1, 3, 2)
        wrapped = np.tile(wr, (1, 1, 8, 1))  # [NC, NBLK, 128, T*8]
        masks = np.stack([(sub == k).astype(np.uint8) for k in (1, 2, 3)],
                         axis=3)  # [NC, NBLK, 128, 3, T]
        masks = masks.reshape(NC, NBLK, 128, 3 * T)
        return np.ascontiguousarray(wrapped), np.ascontiguousarray(masks)

    i2qw, i2mk = quad_arrays(i2g)
    # one blob per block row: rel f32 (4T B) | i2q i16 (16T B) | i2m u8 (3T B)
    bpad = (-23 * T) % 4
    pack = np.concatenate([
        np.ascontiguousarray(rel).view(np.uint8),
        i2qw.view(np.uint8),
        i2mk,
        np.zeros((NC, NBLK, 128, bpad), np.uint8),
    ], axis=3)

    # site props slices (pad rows of zeros)
    sites = np.asarray(inputs["sites"], dtype=np.float32)
    sites_pad = np.zeros((cfg.site_pad, SITE_PROPS), dtype=np.float32)
    sites_pad[:n_sites] = sites
    g2s = np.asarray(inputs["graph_to_sites"])
    gid_pad = np.full(cfg.site_pad, 999.0, dtype=np.float32)
    gid_pad[:n_sites] = g2s.astype(np.float32)

    # constants
    centers = (np.arange(BOND_EXP, dtype=np.float32) * STEP)
    consts = {
        "identity128": np.eye(128, dtype=np.float32),
        "identity128_bf": np.eye(128).astype(BF16),
        "iota_row128": np.tile(np.arange(128, dtype=np.float32), (128, 1)),
        "iota512": np.tile(np.arange(cfg.n_graphs, dtype=np.float32), (128, 1)),
        "centers_row": np.tile(centers, (128, 1)),
        "ones_col": np.ones((128, 1), dtype=BF16),
        "Wse": np.asarray(inputs["W_se"], dtype=np.float32),
        "bse": np.asarray(inputs["b_se"], dtype=np.float32).reshape(SITE_EMB, 1),
        "W1": np.asarray(inputs["W1"], dtype=np.float32),
        "b1": np.asarray(inputs["b1"], dtype=np.float32).reshape(H1, 1),
        "W2": np.asarray(inputs["W2"], dtype=np.float32),
        "b2": np.asarray(inputs["b2"], dtype=np.float32).reshape(H2, 1),
        "W3": np.asarray(inputs["W3"], dtype=np.float32),
        "b3": np.asarray(inputs["b3"], dtype=np.float32).reshape(1, 1),
    }
    Wbe = np.asarray(inputs["W_be"], dtype=np.float32)
    bbe = np.asarray(inputs["b_be"], dtype=np.float32).reshape(-1)
    gate_bias = False
    for L in (1, 2):
        Wsig = np.asarray(inputs[f"W_sig{L}"], dtype=np.float32)
        Wsof = np.asarray(inputs[f"W_sof{L}"], dtype=np.float32)
        bsig = np.asarray(inputs[f"b_sig{L}"], dtype=np.float32).reshape(-1)
        bsof = np.asarray(inputs[f"b_sof{L}"], dtype=np.float32).reshape(-1)
        consts[f"Wa2_{L}"] = np.concatenate(
            [Wsig[0:64], Wsof[0:64]], axis=1).astype(BF16)
        wb = np.concatenate([Wsig[64:128], Wsof[64:128]], axis=1)
        consts[f"Wb2_{L}"] = np.concatenate([wb, wb], axis=0).astype(BF16)
        wc = np.concatenate(
            [Wbe @ Wsig[128:192], Wbe @ Wsof[128:192]], axis=1)
        consts[f"Wc2_{L}"] = np.concatenate([wc, wc], axis=0).astype(BF16)
        zb = np.concatenate([bbe @ Wsig[128:192] + bsig,
                             bbe @ Wsof[128:192] + bsof])
        if np.any(zb != 0):
            gate_bias = True
        consts[f"zbias_{L}"] = np.tile(zb, (128, 1)).astype(np.float32)
    cfg.gate_bias = gate_bias
    if not gate_bias:
        for L in (1, 2):
            del consts[f"zbias_{L}"]

    in_maps = []
    for c in range(NC):
        m = dict(consts)
        m["sites_slice"] = sites_pad[c * R:(c + 1) * R]
        m["gid_blk"] = gid_pad[c * R:(c + 1) * R].reshape(NBLK, 128)
        m["pack_blk"] = pack[c]
        m["bonds_blk"] = bond_blk[c]
        in_maps.append(m)
    return in_maps


def input_specs(cfg):
    NBLK, T, R = cfg.nblk, cfg.T, cfg.R
    specs = {}
    if cfg.gate_bias:
        for L in (1, 2):
            specs[f"zbias_{L}"] = ([128, 128], F32)
    specs.update({
        "sites_slice": ([R, SITE_PROPS], F32),
        "gid_blk": ([NBLK, 128], F32),
        "pack_blk": ([NBLK, 128, 23 * T + ((-23 * T) % 4)], U8),
        "bonds_blk": ([NBLK, 128, T], F32),
        "identity128": ([128, 128], F32),
        "identity128_bf": ([128, 128], BF),
        "iota_row128": ([128, 128], F32),
        "iota512": ([128, cfg.n_graphs], F32),
        "centers_row": ([128, BOND_EXP], F32),
        "ones_col": ([128, 1], BF),
        "Wse": ([SITE_PROPS, SITE_EMB], F32),
        "bse": ([SITE_EMB, 1], F32),
        "Wa2_1": ([SITE_EMB, 128], BF), "Wb2_1": ([128, 128], BF),
        "Wc2_1": ([128, 128], BF),
        "Wa2_2": ([SITE_EMB, 128], BF), "Wb2_2": ([128, 128], BF),
        "Wc2_2": ([128, 128], BF),
        "W1": ([SITE_EMB, H1], F32), "b1": ([H1, 1], F32),
        "W2": ([H1, H2], F32), "b2": ([H2, 1], F32),
        "W3": ([H2, OUT], F32), "b3": ([1, 1], F32),
    })
    return specs


def build_bass(cfg):
    nc = bacc.Bacc("TRN2", target_bir_lowering=False, debug=False,
                   num_devices=cfg.n_cores, num_swdge_queues=4)
    ins = {}
    for name, (shape, dt) in input_specs(cfg).items():
        ins[name] = nc.dram_tensor(name, shape, dt, kind="ExternalInput").ap()
    outs = {
        "out": nc.dram_tensor("out", [cfg.n_graphs, OUT], F32,
                              kind="ExternalOutput").ap()
    }
    with tile.TileContext(nc) as tc:
        build_graph_kernel(nc, tc, ins, outs, cfg)
    nc.compile()
    return nc


_CACHE = {}


def run(inputs, cfg, **kw):
    in_maps = prep_host(inputs, cfg)
    key = (cfg.n_cores, cfg.nblk, cfg.T, cfg.site_pad, cfg.n_graphs,
           cfg.gate_bias, cfg.phases, cfg.no_gather, cfg.no_msg)
    if key not in _CACHE:
        _CACHE[key] = build_bass(cfg)
    nc = _CACHE[key]
    res = run_bass_kernel_spmd(nc, in_maps, core_ids=list(range(cfg.n_cores)), **kw)
    return res


def kernel(**inputs) -> np.ndarray:
    n_sites = inputs["sites"].shape[0]
    cfg = Cfg(n_cores=8, nblk=98, T=None, n_sites=n_sites)
    res = run(inputs, cfg)
    return np.asarray(res.results[0]["out"], dtype=np.float32)


def build_calib(cfg):
    """Same inputs, trivial program — isolates launch+transfer overhead."""
    nc = bacc.Bacc("TRN2", target_bir_lowering=False, debug=False,
                   num_devices=cfg.n_cores)
    for name, (shape, dt) in input_specs(cfg).items():
        nc.dram_tensor(name, shape, dt, kind="ExternalInput").ap()
    out = nc.dram_tensor("out", [cfg.n_graphs, OUT], F32,
                         kind="ExternalOutput").ap()
    with tile.TileContext(nc) as tc:
        with tc.tile_pool(name="sb", bufs=1) as sb:
            t = sb.tile([1, cfg.n_graphs], F32)
            nc.vector.memset(t[:], 0.0)
            nc.sync.dma_start(out[:].rearrange("g o -> o g"), t[:])
    nc.compile()
    return nc
